# revision 33
# baseline (speedup 1.0000x reference)
"""GCN (2-layer) + MLP heads on 8 Trainium2 NeuronCores.

Host: graph preprocessing (self-loops, degree norm, dst-sort, blocking).
Device: dense matmuls in fp32r, message-passing aggregation via
one-hot matmuls over batched indirect-DMA gathers, AllGather for halos.
"""

import math
import os
import sys

import numpy as np

for _p in ("/opt/trn_rl_repo",):
    if _p not in sys.path and os.path.isdir(_p):
        sys.path.insert(0, _p)

from concourse import bacc, bass, mybir, tile  # noqa: E402
from concourse.bass_utils import run_bass_kernel_spmd  # noqa: E402

F32 = mybir.dt.float32
F32R = mybir.dt.float32r
F16 = mybir.dt.float16
I32 = mybir.dt.int32
ALU = mybir.AluOpType
ACTF = mybir.ActivationFunctionType

# ---------------------------------------------------------------- config

N_NODES = 50000
N_EDGES = 800000
NFEAT = 500
NFEAT_P = 512          # K padded to 4x128
H1 = 256               # 2*NHID
H2 = 128               # NHID
PRO = 512              # PROHID
NCLS = 10
NCLS_P = 16
NCORES = 8
SH = N_NODES // NCORES          # 6250 nodes per core
NB = (SH + 127) // 128          # 49 dst blocks per core
SHP = NB * 128                  # 6272 padded rows per core
P = 128

# dtype knobs
TABLE_DT = F16         # dtype of gathered feature tables (xw, hw2)
DENSE_DT = F32R        # dtype used for the dense x@W1 matmul operands
STAGE = 99             # debug: truncate program after stage N (99 = full)


# ---------------------------------------------------------------- host prep

def _preprocess(edge_index):
    """Sort/pad edges; returns per-core index tensors + shared block structure.

    Edges are grouped per 128-node dst block, split by which half of the
    AllGather table their src row falls in (dma_gather idx is int16), and
    padded to shared per-block group counts (Glo[b], Ghi[b])."""
    HALF = NCORES * SHP // 2
    src = np.concatenate([edge_index[0], np.arange(N_NODES, dtype=np.int64)])
    dst = np.concatenate([edge_index[1], np.arange(N_NODES, dtype=np.int64)])
    src = src.astype(np.int64)
    dst = dst.astype(np.int64)

    deg = np.bincount(dst, minlength=N_NODES).astype(np.float64)
    dinv = np.where(deg > 0, 1.0 / np.sqrt(deg), 0.0)
    enorm_all = (dinv[src] * dinv[dst]).astype(np.float32)

    # sort by dst block; one gather table of node PAIRS (idx = row//2)
    agrow_u = ((src // SH) * SHP + (src % SH)).astype(np.int64)
    gblk = (dst // SH) * NB + (dst % SH) // 128
    order = np.argsort(gblk, kind="stable")
    srcs_ag = agrow_u[order]
    dsts = dst[order]
    enorms = enorm_all[order]

    bounds = np.searchsorted(gblk[order], np.arange(NCORES * NB + 1), side="left")
    cnts = np.diff(bounds).reshape(NCORES, NB)

    G = np.maximum(1, (cnts.max(axis=0) + P - 1) // P)
    NG = int(G.sum())
    g0 = np.concatenate([[0], np.cumsum(G)]).astype(np.int64)

    dst_e = np.full((NCORES, NG, P), -1.0, dtype=np.float32)
    dst_o = np.full((NCORES, NG, P), -1.0, dtype=np.float32)
    enormt = np.zeros((NCORES, NG, P), dtype=np.float32)
    idxval = np.zeros((NCORES, NG, P), dtype=np.int16)  # pair row index

    for k in range(NCORES):
        for b in range(NB):
            node_lo = k * SH + b * 128
            i0, i1 = bounds[k * NB + b], bounds[k * NB + b + 1]
            n = i1 - i0
            if n == 0:
                continue
            gi = np.arange(n) // P + g0[b]
            pi = np.arange(n) % P
            doff = (dsts[i0:i1] - node_lo).astype(np.float32)
            par = (srcs_ag[i0:i1] % 2).astype(bool)
            dst_e[k, gi, pi] = np.where(par, -1.0, doff)
            dst_o[k, gi, pi] = np.where(par, doff, -1.0)
            enormt[k, gi, pi] = enorms[i0:i1]
            idxval[k, gi, pi] = (srcs_ag[i0:i1] // 2).astype(np.int16)

    # idx16 [128, 8*NG]: global position N at [N%16, N//16], replicated x8
    idx16 = np.zeros((NCORES, P, 8 * NG), dtype=np.int16)
    iv = idxval.reshape(NCORES, NG * P)
    wrapped = iv.reshape(NCORES, NG, 8, 16).transpose(0, 3, 1, 2).reshape(
        NCORES, 16, NG * 8
    )
    for r in range(8):
        idx16[:, r * 16 : (r + 1) * 16, :] = wrapped

    dst_e = np.ascontiguousarray(dst_e.transpose(0, 2, 1))
    dst_o = np.ascontiguousarray(dst_o.transpose(0, 2, 1))
    enormt = np.ascontiguousarray(enormt.transpose(0, 2, 1))
    return [int(g) for g in G], NG, idx16, dst_e, dst_o, enormt


# ---------------------------------------------------------------- device program

def _build_program(G, NG, num_devices=NCORES):
    """Build the per-core Bass program (SPMD shared across cores)."""
    nc = bacc.Bacc(
        "TRN2", target_bir_lowering=False, debug=False, num_devices=num_devices
    )

    # ---------- I/O
    xt_d = nc.dram_tensor("xt", [NFEAT_P, SHP], DENSE_DT, kind="ExternalInput")
    w1_d = nc.dram_tensor("w1", [NFEAT_P, H1], DENSE_DT, kind="ExternalInput")
    w2_d = nc.dram_tensor("w2", [H1, H2], F16, kind="ExternalInput")
    wi1_d = nc.dram_tensor("wi1", [H2, PRO], F16, kind="ExternalInput")
    wi2_d = nc.dram_tensor("wi2", [PRO, H2], F16, kind="ExternalInput")
    wc1_d = nc.dram_tensor("wc1", [H2, PRO], F16, kind="ExternalInput")
    wc2_d = nc.dram_tensor("wc2", [PRO, NCLS_P], F16, kind="ExternalInput")
    b1b_d = nc.dram_tensor("b1b", [P, H1], F16, kind="ExternalInput")
    b2b_d = nc.dram_tensor("b2b", [P, H2], F16, kind="ExternalInput")
    bi1c_d = nc.dram_tensor("bi1c", [P, 4], F32, kind="ExternalInput")
    bc1c_d = nc.dram_tensor("bc1c", [P, 4], F32, kind="ExternalInput")
    bi2b_d = nc.dram_tensor("bi2b", [P, H2], F32, kind="ExternalInput")
    bc2b_d = nc.dram_tensor("bc2b", [P, NCLS_P], F32, kind="ExternalInput")
    iden_d = nc.dram_tensor("iden", [P, P], F16, kind="ExternalInput")
    iota_d = nc.dram_tensor("iota", [P, P], F16, kind="ExternalInput")
    idx16_d = nc.dram_tensor("idx16", [P, 8 * NG], mybir.dt.int16, kind="ExternalInput")
    dste_d = nc.dram_tensor("dst_e", [P, NG], F32, kind="ExternalInput")
    dsto_d = nc.dram_tensor("dst_o", [P, NG], F32, kind="ExternalInput")
    enrm_d = nc.dram_tensor("enorm", [P, NG], F32, kind="ExternalInput")

    z_out = nc.dram_tensor("z_out", [SHP, H2], F32, kind="ExternalOutput")
    c_out = nc.dram_tensor("c_out", [SHP, NCLS_P], F32, kind="ExternalOutput")

    g0 = [0]
    for g in G:
        g0.append(g0[-1] + g)

    rg = [list(range(num_devices))]

    with tile.TileContext(nc) as tc:
        with (
            tc.tile_pool(name="const", bufs=1) as cpool,
            tc.tile_pool(name="dram", bufs=1, space="DRAM") as dpool,
        ):
            # ---------- DRAM intermediates
            xw_sh = dpool.tile([SHP, H1], TABLE_DT)
            xw_full = dpool.tile([NCORES * SHP, H1], TABLE_DT, addr_space="Shared")
            hw2_sh = dpool.tile([SHP, H2], TABLE_DT)
            hw2_full = dpool.tile([NCORES * SHP, H2], TABLE_DT, addr_space="Shared")

            # ---------- constants into SBUF
            def cload(dram, shape, dt, name):
                t = cpool.tile(shape, dt, name=name)
                nc.sync.dma_start(out=t[:], in_=dram[:])
                return t

            def cload_kt(dram, n_kt, ncols, dt, name):
                # [n_kt*128, ncols] DRAM -> [128, n_kt*ncols] SBUF (k-tiles packed)
                t = cpool.tile([P, n_kt * ncols], dt, name=name)
                for kt in range(n_kt):
                    nc.sync.dma_start(
                        out=t[:, kt * ncols : (kt + 1) * ncols],
                        in_=dram[kt * P : (kt + 1) * P, :],
                    )
                return t

            w1_s = cload_kt(w1_d, 4, H1, DENSE_DT, "w1_s")
            w2_s = cload_kt(w2_d, 2, H2, F16, "w2_s")
            wi1_s = cload(wi1_d, [H2, PRO], F16, "wi1_s")
            wi2_s = cload_kt(wi2_d, 4, H2, F16, "wi2_s")
            wc1_s = cload(wc1_d, [H2, PRO], F16, "wc1_s")
            wc2_s = cload_kt(wc2_d, 4, NCLS_P, F16, "wc2_s")
            b1b_s = cload(b1b_d, [P, H1], F16, "b1b_s")
            b2b_s = cload(b2b_d, [P, H2], F16, "b2b_s")
            bi1c_s = cload(bi1c_d, [P, 4], F32, "bi1c_s")
            bc1c_s = cload(bc1c_d, [P, 4], F32, "bc1c_s")
            bi2b_s = cload(bi2b_d, [P, H2], F32, "bi2b_s")
            bc2b_s = cload(bc2b_d, [P, NCLS_P], F32, "bc2b_s")
            iden_s = cload(iden_d, [P, P], F16, "iden_s")
            iota_s = cload(iota_d, [P, P], F16, "iota_s")
            idx16_s = cload(idx16_d, [P, 8 * NG], mybir.dt.int16, "idx16_s")
            dste_s = cload(dste_d, [P, NG], F32, "dste_s")
            dsto_s = cload(dsto_d, [P, NG], F32, "dsto_s")
            enrm_s = cload(enrm_d, [P, NG], F32, "enrm_s")

            # w1 views: k-tile kt -> [128, H1] at rows kt*128
            # ---------- Phase 1: xw = x @ W1  (fp32r), store fp16
            with (
                tc.tile_pool(name="xpool", bufs=1) as xpool,
                tc.tile_pool(name="p1sb", bufs=3) as p1sb,
                tc.tile_pool(name="p1ps", bufs=2, space="PSUM") as p1ps,
            ):
                xsb = xpool.tile([P, 4 * SHP], DENSE_DT, name="xsb")
                for kt in range(4):
                    nc.sync.dma_start(
                        out=xsb[:, kt * SHP : (kt + 1) * SHP],
                        in_=xt_d[kt * P : (kt + 1) * P, :],
                    )
                for nb in range(NB):
                    ps = p1ps.tile([P, H1], F32, name="xw_ps")
                    for kt in range(4):
                        lhsT = xsb[:, kt * SHP + nb * P : kt * SHP + (nb + 1) * P]
                        nc.tensor.matmul(
                            ps[:],
                            lhsT,
                            w1_s[:, kt * H1 : (kt + 1) * H1],
                            start=(kt == 0),
                            stop=(kt == 3),
                        )
                    xw_t = p1sb.tile([P, H1], TABLE_DT, name="xw_t")
                    nc.scalar.activation(xw_t[:], ps[:], ACTF.Copy)
                    nc.sync.dma_start(
                        out=xw_sh[nb * P : (nb + 1) * P, :], in_=xw_t[:]
                    )

            # ---------- Phase 2: AllGather xw
            if STAGE >= 2:
                nc.gpsimd.collective_compute(
                "AllGather",
                ALU.bypass,
                    replica_groups=rg,
                    ins=[xw_sh[:]],
                    outs=[xw_full[:]],
                )

            # ---------- Phase 3: layer-1 aggregation + ReLU + @W2, store fp16
            if STAGE >= 3:
              with (
                tc.tile_pool(name="p3gt", bufs=8) as p3gt,
                tc.tile_pool(name="p3sb", bufs=3) as p3sb,
                tc.tile_pool(name="p3oh", bufs=4) as p3oh,
                tc.tile_pool(name="p3ps", bufs=2, space="PSUM") as p3ps,
                tc.tile_pool(name="p3pt", bufs=2, space="PSUM") as p3pt,
            ):
                for nb in range(NB):
                    g_lo, g_n = g0[nb], G[nb]
                    ps = p3ps.tile([P, H1], F32, name="agg_ps")
                    gt = p3gt.tile([P, g_n * 2 * H1], TABLE_DT, name="gt", tag="gt")
                    nidx = g_n * P
                    nc.gpsimd.dma_gather(
                        out_ap=gt[:].rearrange("p (j d) -> p j d", d=2 * H1),
                        in_ap=xw_full[:].rearrange("(a b) d -> a (b d)", b=2),
                        idxs_ap=idx16_s[:, 8 * g_lo : 8 * (g_lo + g_n)],
                        num_idxs=nidx,
                        num_idxs_reg=nidx,
                        elem_size=2 * H1,
                        single_packet=False,
                    )
                    for g in range(g_n):
                        for dsel, c0 in ((dste_s, 0), (dsto_s, H1)):
                            oh = p3oh.tile([P, P], TABLE_DT, name="oh", tag="oh")
                            nc.vector.tensor_scalar(
                                out=oh[:],
                                in0=iota_s[:],
                                scalar1=dsel[:, g_lo + g : g_lo + g + 1],
                                scalar2=enrm_s[:, g_lo + g : g_lo + g + 1],
                                op0=ALU.is_equal,
                                op1=ALU.mult,
                            )
                            nc.tensor.matmul(
                                ps[:],
                                oh[:],
                                gt[:, g * 2 * H1 + c0 : g * 2 * H1 + c0 + H1],
                                start=(g == 0 and c0 == 0),
                                stop=False,
                            )
                    nc.tensor.matmul(
                        ps[:], iden_s[:], b1b_s[:], start=False, stop=True
                    )
                    h1 = p3sb.tile([P, H1], TABLE_DT, name="h1", tag="h1")
                    nc.scalar.activation(h1[:], ps[:], ACTF.Relu)
                    # transpose h1 -> h1T (2x 128x128), then @ W2
                    ps2 = p3ps.tile([P, H2], F32, name="hw2_ps", tag="hw2_ps")
                    for half in range(2):
                        pt = p3pt.tile([P, P], TABLE_DT, name="tps", tag="tps")
                        nc.tensor.transpose(
                            pt[:], h1[:, half * P : (half + 1) * P], iden_s[:]
                        )
                        h1t = p3sb.tile([P, P], TABLE_DT, name="h1t", tag="h1t")
                        nc.vector.tensor_copy(h1t[:], pt[:])
                        nc.tensor.matmul(
                            ps2[:],
                            h1t[:],
                            w2_s[:, half * H2 : (half + 1) * H2],
                            start=(half == 0),
                            stop=(half == 1),
                        )
                    hw2t = p3sb.tile([P, H2], TABLE_DT, name="hw2t", tag="hw2t")
                    nc.vector.tensor_copy(hw2t[:], ps2[:])
                    nc.sync.dma_start(
                        out=hw2_sh[nb * P : (nb + 1) * P, :], in_=hw2t[:]
                    )

            # ---------- Phase 4: AllGather hw2
            if STAGE >= 4:
                nc.gpsimd.collective_compute(
                "AllGather",
                ALU.bypass,
                    replica_groups=rg,
                    ins=[hw2_sh[:]],
                    outs=[hw2_full[:]],
                )

            # ---------- Phase 5: layer-2 aggregation -> h2T resident [128, SHP]
            if STAGE >= 5:
              with tc.tile_pool(name="h2pool", bufs=1) as h2pool:
                h2T = h2pool.tile([P, SHP], TABLE_DT, name="h2T")
                with (
                    tc.tile_pool(name="p5gt", bufs=8) as p5gt,
                    tc.tile_pool(name="p5sb", bufs=3) as p5sb,
                    tc.tile_pool(name="p5oh", bufs=4) as p5oh,
                    tc.tile_pool(name="p5ps", bufs=2, space="PSUM") as p5ps,
                    tc.tile_pool(name="p5pt", bufs=2, space="PSUM") as p5pt,
                ):
                    for nb in range(NB):
                        g_lo, g_n = g0[nb], G[nb]
                        ps = p5ps.tile([P, H2], F32, name="agg2_ps")
                        gt = p5gt.tile([P, g_n * 2 * H2], TABLE_DT, name="gt2", tag="gt2")
                        nidx = g_n * P
                        nc.gpsimd.dma_gather(
                            out_ap=gt[:].rearrange("p (j d) -> p j d", d=2 * H2),
                            in_ap=hw2_full[:].rearrange("(a b) d -> a (b d)", b=2),
                            idxs_ap=idx16_s[:, 8 * g_lo : 8 * (g_lo + g_n)],
                            num_idxs=nidx,
                            num_idxs_reg=nidx,
                            elem_size=2 * H2,
                            single_packet=False,
                        )
                        for g in range(g_n):
                            for dsel, c0 in ((dste_s, 0), (dsto_s, H2)):
                                oh = p5oh.tile([P, P], TABLE_DT, name="oh2", tag="oh2")
                                nc.vector.tensor_scalar(
                                    out=oh[:],
                                    in0=iota_s[:],
                                    scalar1=dsel[:, g_lo + g : g_lo + g + 1],
                                    scalar2=enrm_s[:, g_lo + g : g_lo + g + 1],
                                    op0=ALU.is_equal,
                                    op1=ALU.mult,
                                )
                                nc.tensor.matmul(
                                    ps[:],
                                    oh[:],
                                    gt[:, g * 2 * H2 + c0 : g * 2 * H2 + c0 + H2],
                                    start=(g == 0 and c0 == 0),
                                    stop=False,
                                )
                        nc.tensor.matmul(
                            ps[:], iden_s[:], b2b_s[:], start=False, stop=True
                        )
                        h2 = p5sb.tile([P, H2], TABLE_DT, name="h2", tag="h2")
                        nc.scalar.activation(h2[:], ps[:], ACTF.Relu)
                        pt = p5pt.tile([P, P], TABLE_DT, name="tps2", tag="tps2")
                        nc.tensor.transpose(pt[:], h2[:], iden_s[:])
                        nc.vector.tensor_copy(
                            h2T[:, nb * P : (nb + 1) * P], pt[:]
                        )

                # ---------- Phase 6: heads
                chunks = []
                c0 = 0
                while STAGE >= 6 and c0 < SHP:
                    w = min(512, SHP - c0)
                    chunks.append((c0, w))
                    c0 += w
                with (
                    tc.tile_pool(name="p6sb", bufs=3) as p6sb,
                    tc.tile_pool(name="p6ps", bufs=3, space="PSUM") as p6ps,
                    tc.tile_pool(name="p6pz", bufs=2, space="PSUM") as p6pz,
                ):
                    for c0, w in chunks:
                        rhs = h2T[:, c0 : c0 + w]
                        t1T = []
                        u1T = []
                        for i in range(4):
                            pst = p6ps.tile([P, w], F32, name="t1_ps", tag="hps")
                            nc.tensor.matmul(
                                pst[:],
                                wi1_s[:, i * P : (i + 1) * P],
                                rhs,
                                start=True,
                                stop=True,
                            )
                            t1 = p6sb.tile(
                                [P, w], TABLE_DT, name="t1t", tag=f"t1t{i}", bufs=2
                            )
                            nc.scalar.activation(
                                t1[:], pst[:], ACTF.Relu,
                                bias=bi1c_s[:, i : i + 1],
                            )
                            t1T.append(t1)
                        for i in range(4):
                            psu = p6ps.tile([P, w], F32, name="u1_ps", tag="hps")
                            nc.tensor.matmul(
                                psu[:],
                                wc1_s[:, i * P : (i + 1) * P],
                                rhs,
                                start=True,
                                stop=True,
                            )
                            u1 = p6sb.tile(
                                [P, w], TABLE_DT, name="u1t", tag=f"u1t{i}", bufs=2
                            )
                            nc.scalar.activation(
                                u1[:], psu[:], ACTF.Relu,
                                bias=bc1c_s[:, i : i + 1],
                            )
                            u1T.append(u1)
                        for s in range(w // P):
                            row0 = c0 + s * P
                            # ---- z head
                            pz = p6pz.tile([P, H2], F32, name="zi_ps", tag="zps")
                            for i in range(4):
                                nc.tensor.matmul(
                                    pz[:],
                                    t1T[i][:, s * P : (s + 1) * P],
                                    wi2_s[:, i * H2 : (i + 1) * H2],
                                    start=(i == 0),
                                    stop=(i == 3),
                                )
                            zi = p6sb.tile([P, H2], F32, name="zi", tag="zi")
                            nc.vector.tensor_add(zi[:], pz[:], bi2b_s[:])
                            if STAGE >= 8:
                                sq = p6sb.tile([P, H2], F32, name="sq", tag="sq")
                                ss = p6sb.tile([P, 1], F32, name="ss", tag="ss")
                                nc.vector.tensor_mul(sq[:], zi[:], zi[:])
                                nc.vector.tensor_reduce(
                                    out=ss[:], in_=sq[:],
                                    axis=mybir.AxisListType.X, op=ALU.add,
                                )
                                nrm = p6sb.tile([P, 1], F32, name="nrm", tag="nrm")
                                nc.scalar.activation(nrm[:], ss[:], ACTF.Sqrt)
                                nc.vector.tensor_scalar(
                                    out=nrm[:], in0=nrm[:],
                                    scalar1=1e-12, scalar2=None, op0=ALU.max,
                                )
                                rn = p6sb.tile([P, 1], F32, name="rn", tag="rn")
                                nc.vector.reciprocal(rn[:], nrm[:])
                                zt = p6sb.tile([P, H2], F32, name="zt", tag="zt")
                                nc.vector.tensor_scalar(
                                    out=zt[:], in0=zi[:],
                                    scalar1=rn[:], scalar2=None, op0=ALU.mult,
                                )
                            else:
                                zt = zi
                            nc.sync.dma_start(
                                out=z_out[row0 : row0 + P, :], in_=zt[:]
                            )
                            # ---- c head
                            pl = p6pz.tile([P, NCLS_P], F32, name="lg_ps", tag="lps")
                            for i in range(4):
                                nc.tensor.matmul(
                                    pl[:],
                                    u1T[i][:, s * P : (s + 1) * P],
                                    wc2_s[:, i * NCLS_P : (i + 1) * NCLS_P],
                                    start=(i == 0),
                                    stop=(i == 3),
                                )
                            lg = p6sb.tile([P, NCLS_P], F32, name="lg", tag="lg")
                            nc.vector.tensor_add(lg[:], pl[:], bc2b_s[:])
                            if STAGE >= 9:
                                rm = p6sb.tile([P, 1], F32, name="rm", tag="rm")
                                nc.vector.tensor_reduce(
                                    out=rm[:], in_=lg[:],
                                    axis=mybir.AxisListType.X, op=ALU.max,
                                )
                                nrm2 = p6sb.tile([P, 1], F32, name="nrm2", tag="nrm2")
                                nc.vector.tensor_scalar(
                                    out=nrm2[:], in0=rm[:],
                                    scalar1=-1.0, scalar2=None, op0=ALU.mult,
                                )
                                ex = p6sb.tile([P, NCLS_P], F32, name="ex", tag="ex")
                                es = p6sb.tile([P, 1], F32, name="es", tag="es")
                                nc.scalar.activation(
                                    ex[:], lg[:], ACTF.Exp, bias=nrm2[:],
                                )
                                nc.vector.tensor_reduce(
                                    out=es[:], in_=ex[:],
                                    axis=mybir.AxisListType.X, op=ALU.add,
                                )
                                res = p6sb.tile([P, 1], F32, name="res", tag="res")
                                nc.vector.reciprocal(res[:], es[:])
                                ct = p6sb.tile([P, NCLS_P], F32, name="ct", tag="ct")
                                nc.vector.tensor_scalar(
                                    out=ct[:], in0=ex[:],
                                    scalar1=res[:], scalar2=None, op0=ALU.mult,
                                )
                            else:
                                ct = lg
                            nc.sync.dma_start(
                                out=c_out[row0 : row0 + P, :], in_=ct[:]
                            )

            if STAGE < 6:
                _dbg = cpool.tile([P, H2], F32, name="_dbg")
                nc.vector.tensor_copy(_dbg[:], iota_s[:, :H2])
                nc.sync.dma_start(out=z_out[0:P, :], in_=_dbg[:])
                nc.sync.dma_start(out=c_out[0:P, :], in_=_dbg[:, :NCLS_P])

    nc.compile()
    return nc


# ---------------------------------------------------------------- host driver

_CACHE = {}


def _get_program(G, NG):
    key = (tuple(G), NG)
    if key not in _CACHE:
        _CACHE[key] = _build_program(G, NG)
    return _CACHE[key]


def _prepare(x, edge_index, W1, b1, W2, b2, Wi1, bi1, Wi2, bi2, Wc1, bc1, Wc2, bc2):
    x = np.asarray(x, dtype=np.float32)
    edge_index = np.asarray(edge_index)

    G, NG, idx16, dst_e, dst_o, enormt = _preprocess(edge_index)
    nc = _get_program(G, NG)

    # shared (replicated) tensors
    w1 = np.zeros((NFEAT_P, H1), np.float32)
    w1[:NFEAT] = np.asarray(W1, np.float32)
    w2 = np.asarray(W2, np.float16)
    wi1 = np.asarray(Wi1, np.float16)
    wi2 = np.asarray(Wi2, np.float16)
    wc1 = np.asarray(Wc1, np.float16)
    wc2 = np.zeros((PRO, NCLS_P), np.float16)
    wc2[:, :NCLS] = np.asarray(Wc2, np.float16)
    b1b = np.broadcast_to(np.asarray(b1, np.float16), (P, H1)).copy()
    b2b = np.broadcast_to(np.asarray(b2, np.float16), (P, H2)).copy()
    bi1c = np.ascontiguousarray(
        np.broadcast_to(np.asarray(bi1, np.float32).reshape(4, P).T, (P, 4))
    )
    bc1c = np.ascontiguousarray(
        np.broadcast_to(np.asarray(bc1, np.float32).reshape(4, P).T, (P, 4))
    )
    bi2b = np.broadcast_to(np.asarray(bi2, np.float32), (P, H2)).copy()
    bc2b = np.full((P, NCLS_P), -30000.0, np.float32)
    bc2b[:, :NCLS] = np.asarray(bc2, np.float32)
    iden = np.eye(P, dtype=np.float16)
    iota = np.broadcast_to(np.arange(P, dtype=np.float16), (P, P)).copy()

    in_maps = []
    for k in range(NCORES):
        xt = np.zeros((NFEAT_P, SHP), np.float32)
        xt[:NFEAT, :SH] = x[k * SH : (k + 1) * SH].T
        in_maps.append(
            {
                "xt": xt,
                "w1": w1, "w2": w2, "wi1": wi1, "wi2": wi2,
                "wc1": wc1, "wc2": wc2,
                "b1b": b1b, "b2b": b2b, "bi1c": bi1c, "bc1c": bc1c,
                "bi2b": bi2b, "bc2b": bc2b,
                "iden": iden, "iota": iota,
                "idx16": idx16[k], "dst_e": dst_e[k], "dst_o": dst_o[k],
                "enorm": enormt[k],
            }
        )

    return nc, in_maps


def _postprocess(results):
    z = np.concatenate([results[k]["z_out"][:SH] for k in range(NCORES)])
    c = np.concatenate([results[k]["c_out"][:SH, :NCLS] for k in range(NCORES)])
    return z.astype(np.float32), c.astype(np.float32)


def kernel(**inputs):
    nc, in_maps = _prepare(**inputs)
    res = run_bass_kernel_spmd(nc, in_maps, core_ids=list(range(NCORES)))
    kernel.last_results = res
    return _postprocess(res.results)


# revision 34
# speedup vs baseline: 1.1154x; 1.1154x over previous
"""GCN (2-layer) + MLP heads on 8 Trainium2 NeuronCores.

Host: graph preprocessing (self-loops, degree norm, dst-sort, blocking).
Device: dense matmuls in fp32r, message-passing aggregation via
one-hot matmuls over batched indirect-DMA gathers, AllGather for halos.
"""

import math
import os
import sys

import numpy as np

for _p in ("/opt/trn_rl_repo",):
    if _p not in sys.path and os.path.isdir(_p):
        sys.path.insert(0, _p)

from concourse import bacc, bass, mybir, tile  # noqa: E402
from concourse.bass_utils import run_bass_kernel_spmd  # noqa: E402

F32 = mybir.dt.float32
F32R = mybir.dt.float32r
F16 = mybir.dt.float16
I32 = mybir.dt.int32
ALU = mybir.AluOpType
ACTF = mybir.ActivationFunctionType

# ---------------------------------------------------------------- config

N_NODES = 50000
N_EDGES = 800000
NFEAT = 500
NFEAT_P = 512          # K padded to 4x128
H1 = 256               # 2*NHID
H2 = 128               # NHID
PRO = 512              # PROHID
NCLS = 10
NCLS_P = 16
NCORES = 8
SH = N_NODES // NCORES          # 6250 nodes per core
NB = (SH + 127) // 128          # 49 dst blocks per core
SHP = NB * 128                  # 6272 padded rows per core
P = 128

# dtype knobs
TABLE_DT = F16         # dtype of gathered feature tables (xw, hw2)
DENSE_DT = F32R        # dtype used for the dense x@W1 matmul operands
STAGE = 99             # debug: truncate program after stage N (99 = full)


# ---------------------------------------------------------------- host prep

def _preprocess(edge_index):
    """Sort/pad edges; returns per-core index tensors + shared block structure.

    Edges are grouped per 128-node dst block, split by which half of the
    AllGather table their src row falls in (dma_gather idx is int16), and
    padded to shared per-block group counts (Glo[b], Ghi[b])."""
    HALF = NCORES * SHP // 2
    src = np.concatenate([edge_index[0], np.arange(N_NODES, dtype=np.int64)])
    dst = np.concatenate([edge_index[1], np.arange(N_NODES, dtype=np.int64)])
    src = src.astype(np.int64)
    dst = dst.astype(np.int64)

    deg = np.bincount(dst, minlength=N_NODES).astype(np.float64)
    dinv = np.where(deg > 0, 1.0 / np.sqrt(deg), 0.0)
    enorm_all = (dinv[src] * dinv[dst]).astype(np.float32)

    # sort by (dst block, src-half): a block's lo-half edges contiguous,
    # then its hi-half edges (order within a block-half is irrelevant)
    agrow_u = ((src // SH) * SHP + (src % SH)).astype(np.int64)
    gblk = (dst // SH) * NB + (dst % SH) // 128
    keys = gblk * 2 + (agrow_u >= HALF)
    order = np.argsort(keys, kind="stable")
    srcs_ag = agrow_u[order]
    dsts = dst[order]
    enorms = enorm_all[order]

    # per (core, block, half) boundaries
    bound_keys = np.arange(NCORES * NB * 2 + 1)
    bounds = np.searchsorted(keys[order], bound_keys, side="left")
    cnts = np.diff(bounds).reshape(NCORES, NB, 2)   # [core, block, lo/hi]

    Glo = (cnts[:, :, 0].max(axis=0) + P - 1) // P
    Ghi = (cnts[:, :, 1].max(axis=0) + P - 1) // P
    G = np.maximum(1, Glo + Ghi)
    Glo = np.minimum(Glo, G)   # keep G>=1 invariant trivially
    NG = int(G.sum())
    g0 = np.concatenate([[0], np.cumsum(G)]).astype(np.int64)

    dstoff = np.full((NCORES, NG, P), -1.0, dtype=np.float32)
    enormt = np.zeros((NCORES, NG, P), dtype=np.float32)
    idxval = np.zeros((NCORES, NG, P), dtype=np.int16)  # within-half row idx

    for k in range(NCORES):
        for b in range(NB):
            node_lo = k * SH + b * 128
            for h, Gh, goff in ((0, int(Glo[b]), 0), (1, int(Ghi[b]), int(Glo[b]))):
                if Gh == 0:
                    continue
                i0 = bounds[(k * NB + b) * 2 + h]
                i1 = bounds[(k * NB + b) * 2 + h + 1]
                n = i1 - i0
                if n == 0:
                    continue
                gi = np.arange(n) // P + g0[b] + goff
                pi = np.arange(n) % P
                dstoff[k, gi, pi] = (dsts[i0:i1] - node_lo).astype(np.float32)
                enormt[k, gi, pi] = enorms[i0:i1]
                idxval[k, gi, pi] = (srcs_ag[i0:i1] - h * HALF).astype(np.int16)

    # device layouts: dstoff/enorm [128, NG]; idx16 [128, 8*NG] wrapped by 16
    # (gather position n of a call maps to idx tile [n%16, colbase + n//16],
    #  16-row pattern replicated down all 128 partitions)
    idx16 = np.zeros((NCORES, P, 8 * NG), dtype=np.int16)
    iv = idxval.reshape(NCORES, NG * P)
    wrapped = iv.reshape(NCORES, NG, 8, 16).transpose(0, 3, 1, 2).reshape(
        NCORES, 16, NG * 8
    )
    for r in range(8):
        idx16[:, r * 16 : (r + 1) * 16, :] = wrapped

    dstoff = np.ascontiguousarray(dstoff.transpose(0, 2, 1))
    enormt = np.ascontiguousarray(enormt.transpose(0, 2, 1))
    return (
        [int(g) for g in G],
        [int(g) for g in Glo],
        [int(g) for g in Ghi],
        NG,
        idx16,
        dstoff,
        enormt,
    )


# ---------------------------------------------------------------- device program

def _build_program(G, Glo, Ghi, NG, num_devices=NCORES):
    """Build the per-core Bass program (SPMD shared across cores)."""
    nc = bacc.Bacc(
        "TRN2", target_bir_lowering=False, debug=False, num_devices=num_devices
    )

    # ---------- I/O
    xt_d = nc.dram_tensor("xt", [NFEAT_P, SHP], DENSE_DT, kind="ExternalInput")
    w1_d = nc.dram_tensor("w1", [NFEAT_P, H1], DENSE_DT, kind="ExternalInput")
    w2_d = nc.dram_tensor("w2", [H1, H2], F16, kind="ExternalInput")
    wi1_d = nc.dram_tensor("wi1", [H2, PRO], F16, kind="ExternalInput")
    wi2_d = nc.dram_tensor("wi2", [PRO, H2], F16, kind="ExternalInput")
    wc1_d = nc.dram_tensor("wc1", [H2, PRO], F16, kind="ExternalInput")
    wc2_d = nc.dram_tensor("wc2", [PRO, NCLS_P], F16, kind="ExternalInput")
    b1b_d = nc.dram_tensor("b1b", [P, H1], F16, kind="ExternalInput")
    b2b_d = nc.dram_tensor("b2b", [P, H2], F16, kind="ExternalInput")
    bi1c_d = nc.dram_tensor("bi1c", [P, 4], F32, kind="ExternalInput")
    bc1c_d = nc.dram_tensor("bc1c", [P, 4], F32, kind="ExternalInput")
    bi2b_d = nc.dram_tensor("bi2b", [P, H2], F32, kind="ExternalInput")
    bc2b_d = nc.dram_tensor("bc2b", [P, NCLS_P], F32, kind="ExternalInput")
    iden_d = nc.dram_tensor("iden", [P, P], F16, kind="ExternalInput")
    iota_d = nc.dram_tensor("iota", [P, P], F16, kind="ExternalInput")
    idx16_d = nc.dram_tensor("idx16", [P, 8 * NG], mybir.dt.int16, kind="ExternalInput")
    dsto_d = nc.dram_tensor("dstoff", [P, NG], F32, kind="ExternalInput")
    enrm_d = nc.dram_tensor("enorm", [P, NG], F32, kind="ExternalInput")

    z_out = nc.dram_tensor("z_out", [SHP, H2], F32, kind="ExternalOutput")
    c_out = nc.dram_tensor("c_out", [SHP, NCLS_P], F32, kind="ExternalOutput")

    g0 = [0]
    for g in G:
        g0.append(g0[-1] + g)

    rg = [list(range(num_devices))]

    with tile.TileContext(nc) as tc:
        with (
            tc.tile_pool(name="const", bufs=1) as cpool,
            tc.tile_pool(name="dram", bufs=1, space="DRAM") as dpool,
        ):
            # ---------- DRAM intermediates
            xw_sh = dpool.tile([SHP, H1], TABLE_DT)
            xw_full = dpool.tile([NCORES * SHP, H1], TABLE_DT, addr_space="Shared")
            hw2_sh = dpool.tile([SHP, H2], TABLE_DT)
            hw2_full = dpool.tile([NCORES * SHP, H2], TABLE_DT, addr_space="Shared")

            # ---------- constants into SBUF
            def cload(dram, shape, dt, name):
                t = cpool.tile(shape, dt, name=name)
                nc.sync.dma_start(out=t[:], in_=dram[:])
                return t

            def cload_kt(dram, n_kt, ncols, dt, name):
                # [n_kt*128, ncols] DRAM -> [128, n_kt*ncols] SBUF (k-tiles packed)
                t = cpool.tile([P, n_kt * ncols], dt, name=name)
                for kt in range(n_kt):
                    nc.sync.dma_start(
                        out=t[:, kt * ncols : (kt + 1) * ncols],
                        in_=dram[kt * P : (kt + 1) * P, :],
                    )
                return t

            w1_s = cload_kt(w1_d, 4, H1, DENSE_DT, "w1_s")
            w2_s = cload_kt(w2_d, 2, H2, F16, "w2_s")
            wi1_s = cload(wi1_d, [H2, PRO], F16, "wi1_s")
            wi2_s = cload_kt(wi2_d, 4, H2, F16, "wi2_s")
            wc1_s = cload(wc1_d, [H2, PRO], F16, "wc1_s")
            wc2_s = cload_kt(wc2_d, 4, NCLS_P, F16, "wc2_s")
            b1b_s = cload(b1b_d, [P, H1], F16, "b1b_s")
            b2b_s = cload(b2b_d, [P, H2], F16, "b2b_s")
            bi1c_s = cload(bi1c_d, [P, 4], F32, "bi1c_s")
            bc1c_s = cload(bc1c_d, [P, 4], F32, "bc1c_s")
            bi2b_s = cload(bi2b_d, [P, H2], F32, "bi2b_s")
            bc2b_s = cload(bc2b_d, [P, NCLS_P], F32, "bc2b_s")
            iden_s = cload(iden_d, [P, P], F16, "iden_s")
            iota_s = cload(iota_d, [P, P], F16, "iota_s")
            idx16_s = cload(idx16_d, [P, 8 * NG], mybir.dt.int16, "idx16_s")
            dsto_s = cload(dsto_d, [P, NG], F32, "dsto_s")
            enrm_s = cload(enrm_d, [P, NG], F32, "enrm_s")

            # w1 views: k-tile kt -> [128, H1] at rows kt*128
            # ---------- Phase 1: xw = x @ W1  (fp32r), store fp16
            with (
                tc.tile_pool(name="xpool", bufs=1) as xpool,
                tc.tile_pool(name="p1sb", bufs=3) as p1sb,
                tc.tile_pool(name="p1ps", bufs=2, space="PSUM") as p1ps,
            ):
                xsb = xpool.tile([P, 4 * SHP], DENSE_DT, name="xsb")
                for kt in range(4):
                    nc.sync.dma_start(
                        out=xsb[:, kt * SHP : (kt + 1) * SHP],
                        in_=xt_d[kt * P : (kt + 1) * P, :],
                    )
                for nb in range(NB):
                    ps = p1ps.tile([P, H1], F32, name="xw_ps")
                    for kt in range(4):
                        lhsT = xsb[:, kt * SHP + nb * P : kt * SHP + (nb + 1) * P]
                        nc.tensor.matmul(
                            ps[:],
                            lhsT,
                            w1_s[:, kt * H1 : (kt + 1) * H1],
                            start=(kt == 0),
                            stop=(kt == 3),
                        )
                    xw_t = p1sb.tile([P, H1], TABLE_DT, name="xw_t")
                    nc.scalar.activation(xw_t[:], ps[:], ACTF.Copy)
                    nc.sync.dma_start(
                        out=xw_sh[nb * P : (nb + 1) * P, :], in_=xw_t[:]
                    )

            # ---------- Phase 2: AllGather xw
            if STAGE >= 2:
                nc.gpsimd.collective_compute(
                "AllGather",
                ALU.bypass,
                    replica_groups=rg,
                    ins=[xw_sh[:]],
                    outs=[xw_full[:]],
                )

            # ---------- Phase 3: layer-1 aggregation + ReLU + @W2, store fp16
            if STAGE >= 3:
              with (
                tc.tile_pool(name="p3gt", bufs=8) as p3gt,
                tc.tile_pool(name="p3sb", bufs=3) as p3sb,
                tc.tile_pool(name="p3oh", bufs=4) as p3oh,
                tc.tile_pool(name="p3ps", bufs=2, space="PSUM") as p3ps,
                tc.tile_pool(name="p3pt", bufs=2, space="PSUM") as p3pt,
            ):
                HALF = NCORES * SHP // 2
                for nb in range(NB):
                    g_lo, g_n = g0[nb], G[nb]
                    ps = p3ps.tile([P, H1], F32, name="agg_ps")
                    gt = p3gt.tile([P, g_n * H1], TABLE_DT, name="gt", tag="gt")
                    for gh, goff, tab in (
                        (Glo[nb], 0, xw_full[0:HALF, :]),
                        (Ghi[nb], Glo[nb], xw_full[HALF:, :]),
                    ):
                        if gh == 0:
                            continue
                        nidx = gh * P
                        nc.gpsimd.dma_gather(
                            out_ap=gt[:, goff * H1 : (goff + gh) * H1].rearrange(
                                "p (j d) -> p j d", d=H1
                            ),
                            in_ap=tab,
                            idxs_ap=idx16_s[
                                :, 8 * (g_lo + goff) : 8 * (g_lo + goff + gh)
                            ],
                            num_idxs=nidx,
                            num_idxs_reg=nidx,
                            elem_size=H1,
                            single_packet=False,
                        )
                    for g in range(g_n):
                        oh = p3oh.tile([P, P], TABLE_DT, name="oh", tag="oh")
                        nc.vector.tensor_scalar(
                            out=oh[:],
                            in0=iota_s[:],
                            scalar1=dsto_s[:, g_lo + g : g_lo + g + 1],
                            scalar2=enrm_s[:, g_lo + g : g_lo + g + 1],
                            op0=ALU.is_equal,
                            op1=ALU.mult,
                        )
                        nc.tensor.matmul(
                            ps[:],
                            oh[:],
                            gt[:, g * H1 : (g + 1) * H1],
                            start=(g == 0),
                            stop=False,
                        )
                    nc.tensor.matmul(
                        ps[:], iden_s[:], b1b_s[:], start=False, stop=True
                    )
                    h1 = p3sb.tile([P, H1], TABLE_DT, name="h1", tag="h1")
                    nc.scalar.activation(h1[:], ps[:], ACTF.Relu)
                    # transpose h1 -> h1T (2x 128x128), then @ W2
                    ps2 = p3ps.tile([P, H2], F32, name="hw2_ps", tag="hw2_ps")
                    for half in range(2):
                        pt = p3pt.tile([P, P], TABLE_DT, name="tps", tag="tps")
                        nc.tensor.transpose(
                            pt[:], h1[:, half * P : (half + 1) * P], iden_s[:]
                        )
                        h1t = p3sb.tile([P, P], TABLE_DT, name="h1t", tag="h1t")
                        nc.vector.tensor_copy(h1t[:], pt[:])
                        nc.tensor.matmul(
                            ps2[:],
                            h1t[:],
                            w2_s[:, half * H2 : (half + 1) * H2],
                            start=(half == 0),
                            stop=(half == 1),
                        )
                    hw2t = p3sb.tile([P, H2], TABLE_DT, name="hw2t", tag="hw2t")
                    nc.vector.tensor_copy(hw2t[:], ps2[:])
                    nc.sync.dma_start(
                        out=hw2_sh[nb * P : (nb + 1) * P, :], in_=hw2t[:]
                    )

            # ---------- Phase 4: AllGather hw2
            if STAGE >= 4:
                nc.gpsimd.collective_compute(
                "AllGather",
                ALU.bypass,
                    replica_groups=rg,
                    ins=[hw2_sh[:]],
                    outs=[hw2_full[:]],
                )

            # ---------- Phase 5: layer-2 aggregation -> h2T resident [128, SHP]
            if STAGE >= 5:
              with tc.tile_pool(name="h2pool", bufs=1) as h2pool:
                h2T = h2pool.tile([P, SHP], TABLE_DT, name="h2T")
                with (
                    tc.tile_pool(name="p5gt", bufs=8) as p5gt,
                    tc.tile_pool(name="p5sb", bufs=3) as p5sb,
                    tc.tile_pool(name="p5oh", bufs=4) as p5oh,
                    tc.tile_pool(name="p5ps", bufs=2, space="PSUM") as p5ps,
                    tc.tile_pool(name="p5pt", bufs=2, space="PSUM") as p5pt,
                ):
                    HALF2 = NCORES * SHP // 2
                    for nb in range(NB):
                        g_lo, g_n = g0[nb], G[nb]
                        ps = p5ps.tile([P, H2], F32, name="agg2_ps")
                        gt = p5gt.tile([P, g_n * H2], TABLE_DT, name="gt2", tag="gt2")
                        for gh, goff, tab in (
                            (Glo[nb], 0, hw2_full[0:HALF2, :]),
                            (Ghi[nb], Glo[nb], hw2_full[HALF2:, :]),
                        ):
                            if gh == 0:
                                continue
                            nidx = gh * P
                            nc.gpsimd.dma_gather(
                                out_ap=gt[:, goff * H2 : (goff + gh) * H2].rearrange(
                                    "p (j d) -> p j d", d=H2
                                ),
                                in_ap=tab,
                                idxs_ap=idx16_s[
                                    :, 8 * (g_lo + goff) : 8 * (g_lo + goff + gh)
                                ],
                                num_idxs=nidx,
                                num_idxs_reg=nidx,
                                elem_size=H2,
                                single_packet=False,
                            )
                        for g in range(g_n):
                            oh = p5oh.tile([P, P], TABLE_DT, name="oh2", tag="oh2")
                            nc.vector.tensor_scalar(
                                out=oh[:],
                                in0=iota_s[:],
                                scalar1=dsto_s[:, g_lo + g : g_lo + g + 1],
                                scalar2=enrm_s[:, g_lo + g : g_lo + g + 1],
                                op0=ALU.is_equal,
                                op1=ALU.mult,
                            )
                            nc.tensor.matmul(
                                ps[:],
                                oh[:],
                                gt[:, g * H2 : (g + 1) * H2],
                                start=(g == 0),
                                stop=False,
                            )
                        nc.tensor.matmul(
                            ps[:], iden_s[:], b2b_s[:], start=False, stop=True
                        )
                        h2 = p5sb.tile([P, H2], TABLE_DT, name="h2", tag="h2")
                        nc.scalar.activation(h2[:], ps[:], ACTF.Relu)
                        pt = p5pt.tile([P, P], TABLE_DT, name="tps2", tag="tps2")
                        nc.tensor.transpose(pt[:], h2[:], iden_s[:])
                        nc.vector.tensor_copy(
                            h2T[:, nb * P : (nb + 1) * P], pt[:]
                        )

                # ---------- Phase 6: heads
                chunks = []
                c0 = 0
                while STAGE >= 6 and c0 < SHP:
                    w = min(512, SHP - c0)
                    chunks.append((c0, w))
                    c0 += w
                with (
                    tc.tile_pool(name="p6sb", bufs=3) as p6sb,
                    tc.tile_pool(name="p6ps", bufs=3, space="PSUM") as p6ps,
                    tc.tile_pool(name="p6pz", bufs=2, space="PSUM") as p6pz,
                ):
                    for c0, w in chunks:
                        rhs = h2T[:, c0 : c0 + w]
                        t1T = []
                        u1T = []
                        for i in range(4):
                            pst = p6ps.tile([P, w], F32, name="t1_ps", tag="hps")
                            nc.tensor.matmul(
                                pst[:],
                                wi1_s[:, i * P : (i + 1) * P],
                                rhs,
                                start=True,
                                stop=True,
                            )
                            t1 = p6sb.tile(
                                [P, w], TABLE_DT, name="t1t", tag=f"t1t{i}", bufs=2
                            )
                            nc.scalar.activation(
                                t1[:], pst[:], ACTF.Relu,
                                bias=bi1c_s[:, i : i + 1],
                            )
                            t1T.append(t1)
                        for i in range(4):
                            psu = p6ps.tile([P, w], F32, name="u1_ps", tag="hps")
                            nc.tensor.matmul(
                                psu[:],
                                wc1_s[:, i * P : (i + 1) * P],
                                rhs,
                                start=True,
                                stop=True,
                            )
                            u1 = p6sb.tile(
                                [P, w], TABLE_DT, name="u1t", tag=f"u1t{i}", bufs=2
                            )
                            nc.scalar.activation(
                                u1[:], psu[:], ACTF.Relu,
                                bias=bc1c_s[:, i : i + 1],
                            )
                            u1T.append(u1)
                        for s in range(w // P):
                            row0 = c0 + s * P
                            # ---- z head
                            pz = p6pz.tile([P, H2], F32, name="zi_ps", tag="zps")
                            for i in range(4):
                                nc.tensor.matmul(
                                    pz[:],
                                    t1T[i][:, s * P : (s + 1) * P],
                                    wi2_s[:, i * H2 : (i + 1) * H2],
                                    start=(i == 0),
                                    stop=(i == 3),
                                )
                            zi = p6sb.tile([P, H2], F32, name="zi", tag="zi")
                            nc.vector.tensor_add(zi[:], pz[:], bi2b_s[:])
                            if STAGE >= 8:
                                sq = p6sb.tile([P, H2], F32, name="sq", tag="sq")
                                ss = p6sb.tile([P, 1], F32, name="ss", tag="ss")
                                nc.vector.tensor_mul(sq[:], zi[:], zi[:])
                                nc.vector.tensor_reduce(
                                    out=ss[:], in_=sq[:],
                                    axis=mybir.AxisListType.X, op=ALU.add,
                                )
                                nrm = p6sb.tile([P, 1], F32, name="nrm", tag="nrm")
                                nc.scalar.activation(nrm[:], ss[:], ACTF.Sqrt)
                                nc.vector.tensor_scalar(
                                    out=nrm[:], in0=nrm[:],
                                    scalar1=1e-12, scalar2=None, op0=ALU.max,
                                )
                                rn = p6sb.tile([P, 1], F32, name="rn", tag="rn")
                                nc.vector.reciprocal(rn[:], nrm[:])
                                zt = p6sb.tile([P, H2], F32, name="zt", tag="zt")
                                nc.vector.tensor_scalar(
                                    out=zt[:], in0=zi[:],
                                    scalar1=rn[:], scalar2=None, op0=ALU.mult,
                                )
                            else:
                                zt = zi
                            nc.sync.dma_start(
                                out=z_out[row0 : row0 + P, :], in_=zt[:]
                            )
                            # ---- c head
                            pl = p6pz.tile([P, NCLS_P], F32, name="lg_ps", tag="lps")
                            for i in range(4):
                                nc.tensor.matmul(
                                    pl[:],
                                    u1T[i][:, s * P : (s + 1) * P],
                                    wc2_s[:, i * NCLS_P : (i + 1) * NCLS_P],
                                    start=(i == 0),
                                    stop=(i == 3),
                                )
                            lg = p6sb.tile([P, NCLS_P], F32, name="lg", tag="lg")
                            nc.vector.tensor_add(lg[:], pl[:], bc2b_s[:])
                            if STAGE >= 9:
                                rm = p6sb.tile([P, 1], F32, name="rm", tag="rm")
                                nc.vector.tensor_reduce(
                                    out=rm[:], in_=lg[:],
                                    axis=mybir.AxisListType.X, op=ALU.max,
                                )
                                nrm2 = p6sb.tile([P, 1], F32, name="nrm2", tag="nrm2")
                                nc.vector.tensor_scalar(
                                    out=nrm2[:], in0=rm[:],
                                    scalar1=-1.0, scalar2=None, op0=ALU.mult,
                                )
                                ex = p6sb.tile([P, NCLS_P], F32, name="ex", tag="ex")
                                es = p6sb.tile([P, 1], F32, name="es", tag="es")
                                nc.scalar.activation(
                                    ex[:], lg[:], ACTF.Exp, bias=nrm2[:],
                                )
                                nc.vector.tensor_reduce(
                                    out=es[:], in_=ex[:],
                                    axis=mybir.AxisListType.X, op=ALU.add,
                                )
                                res = p6sb.tile([P, 1], F32, name="res", tag="res")
                                nc.vector.reciprocal(res[:], es[:])
                                ct = p6sb.tile([P, NCLS_P], F32, name="ct", tag="ct")
                                nc.vector.tensor_scalar(
                                    out=ct[:], in0=ex[:],
                                    scalar1=res[:], scalar2=None, op0=ALU.mult,
                                )
                            else:
                                ct = lg
                            nc.sync.dma_start(
                                out=c_out[row0 : row0 + P, :], in_=ct[:]
                            )

            if STAGE < 6:
                _dbg = cpool.tile([P, H2], F32, name="_dbg")
                nc.vector.tensor_copy(_dbg[:], iota_s[:, :H2])
                nc.sync.dma_start(out=z_out[0:P, :], in_=_dbg[:])
                nc.sync.dma_start(out=c_out[0:P, :], in_=_dbg[:, :NCLS_P])

    nc.compile()
    return nc


# ---------------------------------------------------------------- host driver

_CACHE = {}


def _get_program(G, Glo, Ghi, NG):
    key = (tuple(Glo), tuple(Ghi), NG)
    if key not in _CACHE:
        _CACHE[key] = _build_program(G, Glo, Ghi, NG)
    return _CACHE[key]


def _prepare(x, edge_index, W1, b1, W2, b2, Wi1, bi1, Wi2, bi2, Wc1, bc1, Wc2, bc2):
    x = np.asarray(x, dtype=np.float32)
    edge_index = np.asarray(edge_index)

    G, Glo, Ghi, NG, idx16, dstoff, enormt = _preprocess(edge_index)
    nc = _get_program(G, Glo, Ghi, NG)

    # shared (replicated) tensors
    w1 = np.zeros((NFEAT_P, H1), np.float32)
    w1[:NFEAT] = np.asarray(W1, np.float32)
    w2 = np.asarray(W2, np.float16)
    wi1 = np.asarray(Wi1, np.float16)
    wi2 = np.asarray(Wi2, np.float16)
    wc1 = np.asarray(Wc1, np.float16)
    wc2 = np.zeros((PRO, NCLS_P), np.float16)
    wc2[:, :NCLS] = np.asarray(Wc2, np.float16)
    b1b = np.broadcast_to(np.asarray(b1, np.float16), (P, H1)).copy()
    b2b = np.broadcast_to(np.asarray(b2, np.float16), (P, H2)).copy()
    bi1c = np.ascontiguousarray(
        np.broadcast_to(np.asarray(bi1, np.float32).reshape(4, P).T, (P, 4))
    )
    bc1c = np.ascontiguousarray(
        np.broadcast_to(np.asarray(bc1, np.float32).reshape(4, P).T, (P, 4))
    )
    bi2b = np.broadcast_to(np.asarray(bi2, np.float32), (P, H2)).copy()
    bc2b = np.full((P, NCLS_P), -30000.0, np.float32)
    bc2b[:, :NCLS] = np.asarray(bc2, np.float32)
    iden = np.eye(P, dtype=np.float16)
    iota = np.broadcast_to(np.arange(P, dtype=np.float16), (P, P)).copy()

    in_maps = []
    for k in range(NCORES):
        xt = np.zeros((NFEAT_P, SHP), np.float32)
        xt[:NFEAT, :SH] = x[k * SH : (k + 1) * SH].T
        in_maps.append(
            {
                "xt": xt,
                "w1": w1, "w2": w2, "wi1": wi1, "wi2": wi2,
                "wc1": wc1, "wc2": wc2,
                "b1b": b1b, "b2b": b2b, "bi1c": bi1c, "bc1c": bc1c,
                "bi2b": bi2b, "bc2b": bc2b,
                "iden": iden, "iota": iota,
                "idx16": idx16[k], "dstoff": dstoff[k], "enorm": enormt[k],
            }
        )

    return nc, in_maps


def _postprocess(results):
    z = np.concatenate([results[k]["z_out"][:SH] for k in range(NCORES)])
    c = np.concatenate([results[k]["c_out"][:SH, :NCLS] for k in range(NCORES)])
    return z.astype(np.float32), c.astype(np.float32)


def kernel(**inputs):
    nc, in_maps = _prepare(**inputs)
    res = run_bass_kernel_spmd(nc, in_maps, core_ids=list(range(NCORES)))
    kernel.last_results = res
    return _postprocess(res.results)


# revision 35
# speedup vs baseline: 1.1424x; 1.0242x over previous
"""GCN (2-layer) + MLP heads on 8 Trainium2 NeuronCores.

Host: graph preprocessing (self-loops, degree norm, dst-sort, blocking).
Device: dense matmuls in fp32r, message-passing aggregation via
one-hot matmuls over batched indirect-DMA gathers, AllGather for halos.
"""

import math
import os
import sys

import numpy as np

for _p in ("/opt/trn_rl_repo",):
    if _p not in sys.path and os.path.isdir(_p):
        sys.path.insert(0, _p)

from concourse import bacc, bass, mybir, tile  # noqa: E402
from concourse.bass_utils import run_bass_kernel_spmd  # noqa: E402

F32 = mybir.dt.float32
F32R = mybir.dt.float32r
F16 = mybir.dt.float16
I32 = mybir.dt.int32
ALU = mybir.AluOpType
ACTF = mybir.ActivationFunctionType

# ---------------------------------------------------------------- config

N_NODES = 50000
N_EDGES = 800000
NFEAT = 500
NFEAT_P = 512          # K padded to 4x128
H1 = 256               # 2*NHID
H2 = 128               # NHID
PRO = 512              # PROHID
NCLS = 10
NCLS_P = 16
NCORES = 8
SH = N_NODES // NCORES          # 6250 nodes per core
NB = (SH + 127) // 128          # 49 dst blocks per core
SHP = NB * 128                  # 6272 padded rows per core
P = 128

# dtype knobs
TABLE_DT = F16         # dtype of gathered feature tables (xw, hw2)
DENSE_DT = F32R        # dtype used for the dense x@W1 matmul operands
STAGE = 99             # debug: truncate program after stage N (99 = full)


# ---------------------------------------------------------------- host prep

def _preprocess(edge_index):
    """Sort/pad edges; returns per-core index tensors + shared block structure.

    Edges are grouped per 128-node dst block, split by which half of the
    AllGather table their src row falls in (dma_gather idx is int16), and
    padded to shared per-block group counts (Glo[b], Ghi[b])."""
    HALF = NCORES * SHP // 2
    src = np.concatenate([edge_index[0], np.arange(N_NODES, dtype=np.int64)])
    dst = np.concatenate([edge_index[1], np.arange(N_NODES, dtype=np.int64)])
    src = src.astype(np.int64)
    dst = dst.astype(np.int64)

    deg = np.bincount(dst, minlength=N_NODES).astype(np.float64)
    dinv = np.where(deg > 0, 1.0 / np.sqrt(deg), 0.0)
    enorm_all = (dinv[src] * dinv[dst]).astype(np.float32)

    # self-loop edges (src==dst) are handled as a per-block diagonal matmul
    # against the core's local table shard -- exclude from the gather stream
    selfm = src == dst
    self_en = np.bincount(
        dst[selfm], weights=enorm_all[selfm].astype(np.float64), minlength=N_NODES
    ).astype(np.float32)
    src, dst, enorm_all = src[~selfm], dst[~selfm], enorm_all[~selfm]

    # sort by (dst block, src-half): a block's lo-half edges contiguous,
    # then its hi-half edges (order within a block-half is irrelevant)
    agrow_u = ((src // SH) * SHP + (src % SH)).astype(np.int64)
    gblk = (dst // SH) * NB + (dst % SH) // 128
    keys = gblk * 2 + (agrow_u >= HALF)
    order = np.argsort(keys, kind="stable")
    srcs_ag = agrow_u[order]
    dsts = dst[order]
    enorms = enorm_all[order]

    # per (core, block, half) boundaries
    bound_keys = np.arange(NCORES * NB * 2 + 1)
    bounds = np.searchsorted(keys[order], bound_keys, side="left")
    cnts = np.diff(bounds).reshape(NCORES, NB, 2)   # [core, block, lo/hi]

    Glo = (cnts[:, :, 0].max(axis=0) + P - 1) // P
    Ghi = (cnts[:, :, 1].max(axis=0) + P - 1) // P
    G = np.maximum(1, Glo + Ghi)
    Glo = np.minimum(Glo, G)   # keep G>=1 invariant trivially
    NG = int(G.sum())
    g0 = np.concatenate([[0], np.cumsum(G)]).astype(np.int64)

    dstoff = np.full((NCORES, NG, P), -1.0, dtype=np.float32)
    enormt = np.zeros((NCORES, NG, P), dtype=np.float32)
    idxval = np.zeros((NCORES, NG, P), dtype=np.int16)  # within-half row idx

    for k in range(NCORES):
        for b in range(NB):
            node_lo = k * SH + b * 128
            for h, Gh, goff in ((0, int(Glo[b]), 0), (1, int(Ghi[b]), int(Glo[b]))):
                if Gh == 0:
                    continue
                i0 = bounds[(k * NB + b) * 2 + h]
                i1 = bounds[(k * NB + b) * 2 + h + 1]
                n = i1 - i0
                if n == 0:
                    continue
                gi = np.arange(n) // P + g0[b] + goff
                pi = np.arange(n) % P
                dstoff[k, gi, pi] = (dsts[i0:i1] - node_lo).astype(np.float32)
                enormt[k, gi, pi] = enorms[i0:i1]
                idxval[k, gi, pi] = (srcs_ag[i0:i1] - h * HALF).astype(np.int16)

    # device layouts: dstoff/enorm [128, NG]; idx16 [128, 8*NG] wrapped by 16
    # (gather position n of a call maps to idx tile [n%16, colbase + n//16],
    #  16-row pattern replicated down all 128 partitions)
    idx16 = np.zeros((NCORES, P, 8 * NG), dtype=np.int16)
    iv = idxval.reshape(NCORES, NG * P)
    wrapped = iv.reshape(NCORES, NG, 8, 16).transpose(0, 3, 1, 2).reshape(
        NCORES, 16, NG * 8
    )
    for r in range(8):
        idx16[:, r * 16 : (r + 1) * 16, :] = wrapped

    # diag columns (one per block) appended after the NG gather columns
    diag_d = np.full((NCORES, NB, P), -1.0, dtype=np.float32)
    diag_e = np.zeros((NCORES, NB, P), dtype=np.float32)
    for k in range(NCORES):
        for b in range(NB):
            node_lo = k * SH + b * 128
            nreal = min(128, SH - b * 128)
            diag_d[k, b, :nreal] = np.arange(nreal, dtype=np.float32)
            diag_e[k, b, :nreal] = self_en[node_lo : node_lo + nreal]
    dstoff = np.concatenate([dstoff, diag_d], axis=1)
    enormt = np.concatenate([enormt, diag_e], axis=1)

    dstoff = np.ascontiguousarray(dstoff.transpose(0, 2, 1))
    enormt = np.ascontiguousarray(enormt.transpose(0, 2, 1))
    return (
        [int(g) for g in G],
        [int(g) for g in Glo],
        [int(g) for g in Ghi],
        NG,
        idx16,
        dstoff,
        enormt,
    )


# ---------------------------------------------------------------- device program

def _build_program(G, Glo, Ghi, NG, num_devices=NCORES):
    """Build the per-core Bass program (SPMD shared across cores)."""
    nc = bacc.Bacc(
        "TRN2", target_bir_lowering=False, debug=False, num_devices=num_devices
    )

    # ---------- I/O
    xt_d = nc.dram_tensor("xt", [NFEAT_P, SHP], DENSE_DT, kind="ExternalInput")
    w1_d = nc.dram_tensor("w1", [NFEAT_P, H1], DENSE_DT, kind="ExternalInput")
    w2_d = nc.dram_tensor("w2", [H1, H2], F16, kind="ExternalInput")
    wi1_d = nc.dram_tensor("wi1", [H2, PRO], F16, kind="ExternalInput")
    wi2_d = nc.dram_tensor("wi2", [PRO, H2], F16, kind="ExternalInput")
    wc1_d = nc.dram_tensor("wc1", [H2, PRO], F16, kind="ExternalInput")
    wc2_d = nc.dram_tensor("wc2", [PRO, NCLS_P], F16, kind="ExternalInput")
    b1b_d = nc.dram_tensor("b1b", [P, H1], F16, kind="ExternalInput")
    b2b_d = nc.dram_tensor("b2b", [P, H2], F16, kind="ExternalInput")
    bi1c_d = nc.dram_tensor("bi1c", [P, 4], F32, kind="ExternalInput")
    bc1c_d = nc.dram_tensor("bc1c", [P, 4], F32, kind="ExternalInput")
    bi2b_d = nc.dram_tensor("bi2b", [P, H2], F32, kind="ExternalInput")
    bc2b_d = nc.dram_tensor("bc2b", [P, NCLS_P], F32, kind="ExternalInput")
    iden_d = nc.dram_tensor("iden", [P, P], F16, kind="ExternalInput")
    iota_d = nc.dram_tensor("iota", [P, P], F16, kind="ExternalInput")
    idx16_d = nc.dram_tensor("idx16", [P, 8 * NG], mybir.dt.int16, kind="ExternalInput")
    dsto_d = nc.dram_tensor("dstoff", [P, NG + NB], F32, kind="ExternalInput")
    enrm_d = nc.dram_tensor("enorm", [P, NG + NB], F32, kind="ExternalInput")

    z_out = nc.dram_tensor("z_out", [SHP, H2], F32, kind="ExternalOutput")
    c_out = nc.dram_tensor("c_out", [SHP, NCLS_P], F32, kind="ExternalOutput")

    g0 = [0]
    for g in G:
        g0.append(g0[-1] + g)

    rg = [list(range(num_devices))]

    with tile.TileContext(nc) as tc:
        with (
            tc.tile_pool(name="const", bufs=1) as cpool,
            tc.tile_pool(name="dram", bufs=1, space="DRAM") as dpool,
        ):
            # ---------- DRAM intermediates
            xw_sh = dpool.tile([SHP, H1], TABLE_DT)
            xw_full = dpool.tile([NCORES * SHP, H1], TABLE_DT, addr_space="Shared")
            hw2_sh = dpool.tile([SHP, H2], TABLE_DT)
            hw2_full = dpool.tile([NCORES * SHP, H2], TABLE_DT, addr_space="Shared")

            # ---------- constants into SBUF
            def cload(dram, shape, dt, name):
                t = cpool.tile(shape, dt, name=name)
                nc.sync.dma_start(out=t[:], in_=dram[:])
                return t

            def cload_kt(dram, n_kt, ncols, dt, name):
                # [n_kt*128, ncols] DRAM -> [128, n_kt*ncols] SBUF (k-tiles packed)
                t = cpool.tile([P, n_kt * ncols], dt, name=name)
                for kt in range(n_kt):
                    nc.sync.dma_start(
                        out=t[:, kt * ncols : (kt + 1) * ncols],
                        in_=dram[kt * P : (kt + 1) * P, :],
                    )
                return t

            w1_s = cload_kt(w1_d, 4, H1, DENSE_DT, "w1_s")
            w2_s = cload_kt(w2_d, 2, H2, F16, "w2_s")
            wi1_s = cload(wi1_d, [H2, PRO], F16, "wi1_s")
            wi2_s = cload_kt(wi2_d, 4, H2, F16, "wi2_s")
            wc1_s = cload(wc1_d, [H2, PRO], F16, "wc1_s")
            wc2_s = cload_kt(wc2_d, 4, NCLS_P, F16, "wc2_s")
            b1b_s = cload(b1b_d, [P, H1], F16, "b1b_s")
            b2b_s = cload(b2b_d, [P, H2], F16, "b2b_s")
            bi1c_s = cload(bi1c_d, [P, 4], F32, "bi1c_s")
            bc1c_s = cload(bc1c_d, [P, 4], F32, "bc1c_s")
            bi2b_s = cload(bi2b_d, [P, H2], F32, "bi2b_s")
            bc2b_s = cload(bc2b_d, [P, NCLS_P], F32, "bc2b_s")
            iden_s = cload(iden_d, [P, P], F16, "iden_s")
            iota_s = cload(iota_d, [P, P], F16, "iota_s")
            idx16_s = cload(idx16_d, [P, 8 * NG], mybir.dt.int16, "idx16_s")
            dsto_s = cload(dsto_d, [P, NG + NB], F32, "dsto_s")
            enrm_s = cload(enrm_d, [P, NG + NB], F32, "enrm_s")

            # w1 views: k-tile kt -> [128, H1] at rows kt*128
            # ---------- Phase 1: xw = x @ W1  (fp32r), store fp16
            with (
                tc.tile_pool(name="xpool", bufs=1) as xpool,
                tc.tile_pool(name="p1sb", bufs=3) as p1sb,
                tc.tile_pool(name="p1ps", bufs=2, space="PSUM") as p1ps,
            ):
                xsb = xpool.tile([P, 4 * SHP], DENSE_DT, name="xsb")
                for kt in range(4):
                    nc.sync.dma_start(
                        out=xsb[:, kt * SHP : (kt + 1) * SHP],
                        in_=xt_d[kt * P : (kt + 1) * P, :],
                    )
                for nb in range(NB):
                    ps = p1ps.tile([P, H1], F32, name="xw_ps")
                    for kt in range(4):
                        lhsT = xsb[:, kt * SHP + nb * P : kt * SHP + (nb + 1) * P]
                        nc.tensor.matmul(
                            ps[:],
                            lhsT,
                            w1_s[:, kt * H1 : (kt + 1) * H1],
                            start=(kt == 0),
                            stop=(kt == 3),
                        )
                    xw_t = p1sb.tile([P, H1], TABLE_DT, name="xw_t")
                    nc.scalar.activation(xw_t[:], ps[:], ACTF.Copy)
                    nc.sync.dma_start(
                        out=xw_sh[nb * P : (nb + 1) * P, :], in_=xw_t[:]
                    )

            # ---------- Phase 2: AllGather xw
            if STAGE >= 2:
                nc.gpsimd.collective_compute(
                "AllGather",
                ALU.bypass,
                    replica_groups=rg,
                    ins=[xw_sh[:]],
                    outs=[xw_full[:]],
                )

            # ---------- Phase 3: layer-1 aggregation + ReLU + @W2, store fp16
            if STAGE >= 3:
              with (
                tc.tile_pool(name="p3gt", bufs=8) as p3gt,
                tc.tile_pool(name="p3sb", bufs=3) as p3sb,
                tc.tile_pool(name="p3oh", bufs=4) as p3oh,
                tc.tile_pool(name="p3ps", bufs=2, space="PSUM") as p3ps,
                tc.tile_pool(name="p3pt", bufs=2, space="PSUM") as p3pt,
            ):
                HALF = NCORES * SHP // 2
                for nb in range(NB):
                    g_lo, g_n = g0[nb], G[nb]
                    ps = p3ps.tile([P, H1], F32, name="agg_ps")
                    gt = p3gt.tile([P, g_n * H1], TABLE_DT, name="gt", tag="gt")
                    for gh, goff, tab in (
                        (Glo[nb], 0, xw_full[0:HALF, :]),
                        (Ghi[nb], Glo[nb], xw_full[HALF:, :]),
                    ):
                        if gh == 0:
                            continue
                        nidx = gh * P
                        nc.gpsimd.dma_gather(
                            out_ap=gt[:, goff * H1 : (goff + gh) * H1].rearrange(
                                "p (j d) -> p j d", d=H1
                            ),
                            in_ap=tab,
                            idxs_ap=idx16_s[
                                :, 8 * (g_lo + goff) : 8 * (g_lo + goff + gh)
                            ],
                            num_idxs=nidx,
                            num_idxs_reg=nidx,
                            elem_size=H1,
                            single_packet=False,
                        )
                    for g in range(g_n):
                        oh = p3oh.tile([P, P], TABLE_DT, name="oh", tag="oh")
                        nc.vector.tensor_scalar(
                            out=oh[:],
                            in0=iota_s[:],
                            scalar1=dsto_s[:, g_lo + g : g_lo + g + 1],
                            scalar2=enrm_s[:, g_lo + g : g_lo + g + 1],
                            op0=ALU.is_equal,
                            op1=ALU.mult,
                        )
                        nc.tensor.matmul(
                            ps[:],
                            oh[:],
                            gt[:, g * H1 : (g + 1) * H1],
                            start=(g == 0),
                            stop=False,
                        )
                    gtd = p3gt.tile([P, H1], TABLE_DT, name="gtd", tag="gtd")
                    nc.sync.dma_start(
                        out=gtd[:], in_=xw_sh[nb * P : (nb + 1) * P, :]
                    )
                    ohd = p3oh.tile([P, P], TABLE_DT, name="ohd", tag="oh")
                    nc.vector.tensor_scalar(
                        out=ohd[:],
                        in0=iota_s[:],
                        scalar1=dsto_s[:, NG + nb : NG + nb + 1],
                        scalar2=enrm_s[:, NG + nb : NG + nb + 1],
                        op0=ALU.is_equal,
                        op1=ALU.mult,
                    )
                    nc.tensor.matmul(ps[:], ohd[:], gtd[:], start=False, stop=False)
                    nc.tensor.matmul(
                        ps[:], iden_s[:], b1b_s[:], start=False, stop=True
                    )
                    h1 = p3sb.tile([P, H1], TABLE_DT, name="h1", tag="h1")
                    nc.scalar.activation(h1[:], ps[:], ACTF.Relu)
                    # transpose h1 -> h1T (2x 128x128), then @ W2
                    ps2 = p3ps.tile([P, H2], F32, name="hw2_ps", tag="hw2_ps")
                    for half in range(2):
                        pt = p3pt.tile([P, P], TABLE_DT, name="tps", tag="tps")
                        nc.tensor.transpose(
                            pt[:], h1[:, half * P : (half + 1) * P], iden_s[:]
                        )
                        h1t = p3sb.tile([P, P], TABLE_DT, name="h1t", tag="h1t")
                        nc.vector.tensor_copy(h1t[:], pt[:])
                        nc.tensor.matmul(
                            ps2[:],
                            h1t[:],
                            w2_s[:, half * H2 : (half + 1) * H2],
                            start=(half == 0),
                            stop=(half == 1),
                        )
                    hw2t = p3sb.tile([P, H2], TABLE_DT, name="hw2t", tag="hw2t")
                    nc.vector.tensor_copy(hw2t[:], ps2[:])
                    nc.sync.dma_start(
                        out=hw2_sh[nb * P : (nb + 1) * P, :], in_=hw2t[:]
                    )

            # ---------- Phase 4: AllGather hw2
            if STAGE >= 4:
                nc.gpsimd.collective_compute(
                "AllGather",
                ALU.bypass,
                    replica_groups=rg,
                    ins=[hw2_sh[:]],
                    outs=[hw2_full[:]],
                )

            # ---------- Phase 5: layer-2 aggregation -> h2T resident [128, SHP]
            if STAGE >= 5:
              with tc.tile_pool(name="h2pool", bufs=1) as h2pool:
                h2T = h2pool.tile([P, SHP], TABLE_DT, name="h2T")
                with (
                    tc.tile_pool(name="p5gt", bufs=8) as p5gt,
                    tc.tile_pool(name="p5sb", bufs=3) as p5sb,
                    tc.tile_pool(name="p5oh", bufs=4) as p5oh,
                    tc.tile_pool(name="p5ps", bufs=2, space="PSUM") as p5ps,
                    tc.tile_pool(name="p5pt", bufs=2, space="PSUM") as p5pt,
                ):
                    HALF2 = NCORES * SHP // 2
                    for nb in range(NB):
                        g_lo, g_n = g0[nb], G[nb]
                        ps = p5ps.tile([P, H2], F32, name="agg2_ps")
                        gt = p5gt.tile([P, g_n * H2], TABLE_DT, name="gt2", tag="gt2")
                        for gh, goff, tab in (
                            (Glo[nb], 0, hw2_full[0:HALF2, :]),
                            (Ghi[nb], Glo[nb], hw2_full[HALF2:, :]),
                        ):
                            if gh == 0:
                                continue
                            nidx = gh * P
                            nc.gpsimd.dma_gather(
                                out_ap=gt[:, goff * H2 : (goff + gh) * H2].rearrange(
                                    "p (j d) -> p j d", d=H2
                                ),
                                in_ap=tab,
                                idxs_ap=idx16_s[
                                    :, 8 * (g_lo + goff) : 8 * (g_lo + goff + gh)
                                ],
                                num_idxs=nidx,
                                num_idxs_reg=nidx,
                                elem_size=H2,
                                single_packet=False,
                            )
                        for g in range(g_n):
                            oh = p5oh.tile([P, P], TABLE_DT, name="oh2", tag="oh2")
                            nc.vector.tensor_scalar(
                                out=oh[:],
                                in0=iota_s[:],
                                scalar1=dsto_s[:, g_lo + g : g_lo + g + 1],
                                scalar2=enrm_s[:, g_lo + g : g_lo + g + 1],
                                op0=ALU.is_equal,
                                op1=ALU.mult,
                            )
                            nc.tensor.matmul(
                                ps[:],
                                oh[:],
                                gt[:, g * H2 : (g + 1) * H2],
                                start=(g == 0),
                                stop=False,
                            )
                        gtd = p5gt.tile([P, H2], TABLE_DT, name="gtd2", tag="gtd2")
                        nc.sync.dma_start(
                            out=gtd[:], in_=hw2_sh[nb * P : (nb + 1) * P, :]
                        )
                        ohd = p5oh.tile([P, P], TABLE_DT, name="ohd2", tag="oh2")
                        nc.vector.tensor_scalar(
                            out=ohd[:],
                            in0=iota_s[:],
                            scalar1=dsto_s[:, NG + nb : NG + nb + 1],
                            scalar2=enrm_s[:, NG + nb : NG + nb + 1],
                            op0=ALU.is_equal,
                            op1=ALU.mult,
                        )
                        nc.tensor.matmul(ps[:], ohd[:], gtd[:], start=False, stop=False)
                        nc.tensor.matmul(
                            ps[:], iden_s[:], b2b_s[:], start=False, stop=True
                        )
                        h2 = p5sb.tile([P, H2], TABLE_DT, name="h2", tag="h2")
                        nc.scalar.activation(h2[:], ps[:], ACTF.Relu)
                        pt = p5pt.tile([P, P], TABLE_DT, name="tps2", tag="tps2")
                        nc.tensor.transpose(pt[:], h2[:], iden_s[:])
                        nc.vector.tensor_copy(
                            h2T[:, nb * P : (nb + 1) * P], pt[:]
                        )

                # ---------- Phase 6: heads
                chunks = []
                c0 = 0
                while STAGE >= 6 and c0 < SHP:
                    w = min(512, SHP - c0)
                    chunks.append((c0, w))
                    c0 += w
                with (
                    tc.tile_pool(name="p6sb", bufs=3) as p6sb,
                    tc.tile_pool(name="p6ps", bufs=3, space="PSUM") as p6ps,
                    tc.tile_pool(name="p6pz", bufs=2, space="PSUM") as p6pz,
                ):
                    for c0, w in chunks:
                        rhs = h2T[:, c0 : c0 + w]
                        t1T = []
                        u1T = []
                        for i in range(4):
                            pst = p6ps.tile([P, w], F32, name="t1_ps", tag="hps")
                            nc.tensor.matmul(
                                pst[:],
                                wi1_s[:, i * P : (i + 1) * P],
                                rhs,
                                start=True,
                                stop=True,
                            )
                            t1 = p6sb.tile(
                                [P, w], TABLE_DT, name="t1t", tag=f"t1t{i}", bufs=2
                            )
                            nc.scalar.activation(
                                t1[:], pst[:], ACTF.Relu,
                                bias=bi1c_s[:, i : i + 1],
                            )
                            t1T.append(t1)
                        for i in range(4):
                            psu = p6ps.tile([P, w], F32, name="u1_ps", tag="hps")
                            nc.tensor.matmul(
                                psu[:],
                                wc1_s[:, i * P : (i + 1) * P],
                                rhs,
                                start=True,
                                stop=True,
                            )
                            u1 = p6sb.tile(
                                [P, w], TABLE_DT, name="u1t", tag=f"u1t{i}", bufs=2
                            )
                            nc.scalar.activation(
                                u1[:], psu[:], ACTF.Relu,
                                bias=bc1c_s[:, i : i + 1],
                            )
                            u1T.append(u1)
                        for s in range(w // P):
                            row0 = c0 + s * P
                            # ---- z head
                            pz = p6pz.tile([P, H2], F32, name="zi_ps", tag="zps")
                            for i in range(4):
                                nc.tensor.matmul(
                                    pz[:],
                                    t1T[i][:, s * P : (s + 1) * P],
                                    wi2_s[:, i * H2 : (i + 1) * H2],
                                    start=(i == 0),
                                    stop=(i == 3),
                                )
                            zi = p6sb.tile([P, H2], F32, name="zi", tag="zi")
                            nc.vector.tensor_add(zi[:], pz[:], bi2b_s[:])
                            if STAGE >= 8:
                                sq = p6sb.tile([P, H2], F32, name="sq", tag="sq")
                                ss = p6sb.tile([P, 1], F32, name="ss", tag="ss")
                                nc.vector.tensor_mul(sq[:], zi[:], zi[:])
                                nc.vector.tensor_reduce(
                                    out=ss[:], in_=sq[:],
                                    axis=mybir.AxisListType.X, op=ALU.add,
                                )
                                nrm = p6sb.tile([P, 1], F32, name="nrm", tag="nrm")
                                nc.scalar.activation(nrm[:], ss[:], ACTF.Sqrt)
                                nc.vector.tensor_scalar(
                                    out=nrm[:], in0=nrm[:],
                                    scalar1=1e-12, scalar2=None, op0=ALU.max,
                                )
                                rn = p6sb.tile([P, 1], F32, name="rn", tag="rn")
                                nc.vector.reciprocal(rn[:], nrm[:])
                                zt = p6sb.tile([P, H2], F32, name="zt", tag="zt")
                                nc.vector.tensor_scalar(
                                    out=zt[:], in0=zi[:],
                                    scalar1=rn[:], scalar2=None, op0=ALU.mult,
                                )
                            else:
                                zt = zi
                            nc.sync.dma_start(
                                out=z_out[row0 : row0 + P, :], in_=zt[:]
                            )
                            # ---- c head
                            pl = p6pz.tile([P, NCLS_P], F32, name="lg_ps", tag="lps")
                            for i in range(4):
                                nc.tensor.matmul(
                                    pl[:],
                                    u1T[i][:, s * P : (s + 1) * P],
                                    wc2_s[:, i * NCLS_P : (i + 1) * NCLS_P],
                                    start=(i == 0),
                                    stop=(i == 3),
                                )
                            lg = p6sb.tile([P, NCLS_P], F32, name="lg", tag="lg")
                            nc.vector.tensor_add(lg[:], pl[:], bc2b_s[:])
                            if STAGE >= 9:
                                rm = p6sb.tile([P, 1], F32, name="rm", tag="rm")
                                nc.vector.tensor_reduce(
                                    out=rm[:], in_=lg[:],
                                    axis=mybir.AxisListType.X, op=ALU.max,
                                )
                                nrm2 = p6sb.tile([P, 1], F32, name="nrm2", tag="nrm2")
                                nc.vector.tensor_scalar(
                                    out=nrm2[:], in0=rm[:],
                                    scalar1=-1.0, scalar2=None, op0=ALU.mult,
                                )
                                ex = p6sb.tile([P, NCLS_P], F32, name="ex", tag="ex")
                                es = p6sb.tile([P, 1], F32, name="es", tag="es")
                                nc.scalar.activation(
                                    ex[:], lg[:], ACTF.Exp, bias=nrm2[:],
                                )
                                nc.vector.tensor_reduce(
                                    out=es[:], in_=ex[:],
                                    axis=mybir.AxisListType.X, op=ALU.add,
                                )
                                res = p6sb.tile([P, 1], F32, name="res", tag="res")
                                nc.vector.reciprocal(res[:], es[:])
                                ct = p6sb.tile([P, NCLS_P], F32, name="ct", tag="ct")
                                nc.vector.tensor_scalar(
                                    out=ct[:], in0=ex[:],
                                    scalar1=res[:], scalar2=None, op0=ALU.mult,
                                )
                            else:
                                ct = lg
                            nc.sync.dma_start(
                                out=c_out[row0 : row0 + P, :], in_=ct[:]
                            )

            if STAGE < 6:
                _dbg = cpool.tile([P, H2], F32, name="_dbg")
                nc.vector.tensor_copy(_dbg[:], iota_s[:, :H2])
                nc.sync.dma_start(out=z_out[0:P, :], in_=_dbg[:])
                nc.sync.dma_start(out=c_out[0:P, :], in_=_dbg[:, :NCLS_P])

    nc.compile()
    return nc


# ---------------------------------------------------------------- host driver

_CACHE = {}


def _get_program(G, Glo, Ghi, NG):
    key = (tuple(Glo), tuple(Ghi), NG)
    if key not in _CACHE:
        _CACHE[key] = _build_program(G, Glo, Ghi, NG)
    return _CACHE[key]


def _prepare(x, edge_index, W1, b1, W2, b2, Wi1, bi1, Wi2, bi2, Wc1, bc1, Wc2, bc2):
    x = np.asarray(x, dtype=np.float32)
    edge_index = np.asarray(edge_index)

    G, Glo, Ghi, NG, idx16, dstoff, enormt = _preprocess(edge_index)
    nc = _get_program(G, Glo, Ghi, NG)

    # shared (replicated) tensors
    w1 = np.zeros((NFEAT_P, H1), np.float32)
    w1[:NFEAT] = np.asarray(W1, np.float32)
    w2 = np.asarray(W2, np.float16)
    wi1 = np.asarray(Wi1, np.float16)
    wi2 = np.asarray(Wi2, np.float16)
    wc1 = np.asarray(Wc1, np.float16)
    wc2 = np.zeros((PRO, NCLS_P), np.float16)
    wc2[:, :NCLS] = np.asarray(Wc2, np.float16)
    b1b = np.broadcast_to(np.asarray(b1, np.float16), (P, H1)).copy()
    b2b = np.broadcast_to(np.asarray(b2, np.float16), (P, H2)).copy()
    bi1c = np.ascontiguousarray(
        np.broadcast_to(np.asarray(bi1, np.float32).reshape(4, P).T, (P, 4))
    )
    bc1c = np.ascontiguousarray(
        np.broadcast_to(np.asarray(bc1, np.float32).reshape(4, P).T, (P, 4))
    )
    bi2b = np.broadcast_to(np.asarray(bi2, np.float32), (P, H2)).copy()
    bc2b = np.full((P, NCLS_P), -30000.0, np.float32)
    bc2b[:, :NCLS] = np.asarray(bc2, np.float32)
    iden = np.eye(P, dtype=np.float16)
    iota = np.broadcast_to(np.arange(P, dtype=np.float16), (P, P)).copy()

    in_maps = []
    for k in range(NCORES):
        xt = np.zeros((NFEAT_P, SHP), np.float32)
        xt[:NFEAT, :SH] = x[k * SH : (k + 1) * SH].T
        in_maps.append(
            {
                "xt": xt,
                "w1": w1, "w2": w2, "wi1": wi1, "wi2": wi2,
                "wc1": wc1, "wc2": wc2,
                "b1b": b1b, "b2b": b2b, "bi1c": bi1c, "bc1c": bc1c,
                "bi2b": bi2b, "bc2b": bc2b,
                "iden": iden, "iota": iota,
                "idx16": idx16[k], "dstoff": dstoff[k], "enorm": enormt[k],
            }
        )

    return nc, in_maps


def _postprocess(results):
    z = np.concatenate([results[k]["z_out"][:SH] for k in range(NCORES)])
    c = np.concatenate([results[k]["c_out"][:SH, :NCLS] for k in range(NCORES)])
    return z.astype(np.float32), c.astype(np.float32)


def kernel(**inputs):
    nc, in_maps = _prepare(**inputs)
    res = run_bass_kernel_spmd(nc, in_maps, core_ids=list(range(NCORES)))
    kernel.last_results = res
    return _postprocess(res.results)


# revision 36
# speedup vs baseline: 1.2326x; 1.0789x over previous
"""GCN (2-layer) + MLP heads on 8 Trainium2 NeuronCores.

Host: graph preprocessing (self-loops, degree norm, dst-sort, blocking).
Device: dense matmuls in fp32r, message-passing aggregation via
one-hot matmuls over batched indirect-DMA gathers, AllGather for halos.
"""

import math
import os
import sys

import numpy as np

for _p in ("/opt/trn_rl_repo",):
    if _p not in sys.path and os.path.isdir(_p):
        sys.path.insert(0, _p)

from concourse import bacc, bass, mybir, tile  # noqa: E402
from concourse.bass_utils import run_bass_kernel_spmd  # noqa: E402

F32 = mybir.dt.float32
F32R = mybir.dt.float32r
F16 = mybir.dt.float16
I32 = mybir.dt.int32
ALU = mybir.AluOpType
ACTF = mybir.ActivationFunctionType

# ---------------------------------------------------------------- config

N_NODES = 50000
N_EDGES = 800000
NFEAT = 500
NFEAT_P = 512          # K padded to 4x128
H1 = 256               # 2*NHID
H2 = 128               # NHID
PRO = 512              # PROHID
NCLS = 10
NCLS_P = 16
NCORES = 8
SH = N_NODES // NCORES          # 6250 nodes per core
NB = (SH + 127) // 128          # 49 dst blocks per core
SHP = NB * 128                  # 6272 padded rows per core
P = 128

# dtype knobs
TABLE_DT = F16         # dtype of gathered feature tables (xw, hw2)
DENSE_DT = F32R        # dtype used for the dense x@W1 matmul operands
STAGE = 99             # debug: truncate program after stage N (99 = full)


# ---------------------------------------------------------------- host prep

def _preprocess(edge_index):
    """Sort/pad edges; returns per-core index tensors + shared block structure.

    Edges are grouped per 128-node dst block, split by which half of the
    AllGather table their src row falls in (dma_gather idx is int16), and
    padded to shared per-block group counts (Glo[b], Ghi[b])."""
    HALF = NCORES * SHP // 2
    src = np.concatenate([edge_index[0], np.arange(N_NODES, dtype=np.int64)])
    dst = np.concatenate([edge_index[1], np.arange(N_NODES, dtype=np.int64)])
    src = src.astype(np.int64)
    dst = dst.astype(np.int64)

    deg = np.bincount(dst, minlength=N_NODES).astype(np.float64)
    dinv = np.where(deg > 0, 1.0 / np.sqrt(deg), 0.0)
    enorm_all = (dinv[src] * dinv[dst]).astype(np.float32)

    # self-loop edges (src==dst) are handled as a per-block diagonal matmul
    # against the core's local table shard -- exclude from the gather stream
    selfm = src == dst
    self_en = np.bincount(
        dst[selfm], weights=enorm_all[selfm].astype(np.float64), minlength=N_NODES
    ).astype(np.float32)
    src, dst, enorm_all = src[~selfm], dst[~selfm], enorm_all[~selfm]

    # sort by (dst block, src-half): a block's lo-half edges contiguous,
    # then its hi-half edges (order within a block-half is irrelevant)
    agrow_u = ((src // SH) * SHP + (src % SH)).astype(np.int64)
    gblk = (dst // SH) * NB + (dst % SH) // 128
    keys = gblk * 2 + (agrow_u >= HALF)
    order = np.argsort(keys, kind="stable")
    srcs_ag = agrow_u[order]
    dsts = dst[order]
    enorms = enorm_all[order]

    # per (core, block, half) boundaries
    bound_keys = np.arange(NCORES * NB * 2 + 1)
    bounds = np.searchsorted(keys[order], bound_keys, side="left")
    cnts = np.diff(bounds).reshape(NCORES, NB, 2)   # [core, block, lo/hi]

    Glo = (cnts[:, :, 0].max(axis=0) + P - 1) // P
    Ghi = (cnts[:, :, 1].max(axis=0) + P - 1) // P
    G = np.maximum(1, Glo + Ghi)
    Glo = np.minimum(Glo, G)   # keep G>=1 invariant trivially
    NG = int(G.sum())
    g0 = np.concatenate([[0], np.cumsum(G)]).astype(np.int64)

    dstoff = np.full((NCORES, NG, P), -1.0, dtype=np.float32)
    enormt = np.zeros((NCORES, NG, P), dtype=np.float32)
    idxval = np.zeros((NCORES, NG, P), dtype=np.int16)  # within-half row idx

    for k in range(NCORES):
        for b in range(NB):
            node_lo = k * SH + b * 128
            for h, Gh, goff in ((0, int(Glo[b]), 0), (1, int(Ghi[b]), int(Glo[b]))):
                if Gh == 0:
                    continue
                i0 = bounds[(k * NB + b) * 2 + h]
                i1 = bounds[(k * NB + b) * 2 + h + 1]
                n = i1 - i0
                if n == 0:
                    continue
                gi = np.arange(n) // P + g0[b] + goff
                pi = np.arange(n) % P
                dstoff[k, gi, pi] = (dsts[i0:i1] - node_lo).astype(np.float32)
                enormt[k, gi, pi] = enorms[i0:i1]
                idxval[k, gi, pi] = (srcs_ag[i0:i1] - h * HALF).astype(np.int16)

    # device layouts: dstoff/enorm [128, NG]; idx16 [128, 8*NG] wrapped by 16
    # (gather position n of a call maps to idx tile [n%16, colbase + n//16],
    #  16-row pattern replicated down all 128 partitions)
    idx16 = np.zeros((NCORES, P, 8 * NG), dtype=np.int16)
    iv = idxval.reshape(NCORES, NG * P)
    wrapped = iv.reshape(NCORES, NG, 8, 16).transpose(0, 3, 1, 2).reshape(
        NCORES, 16, NG * 8
    )
    for r in range(8):
        idx16[:, r * 16 : (r + 1) * 16, :] = wrapped

    # diag columns (one per block) appended after the NG gather columns
    diag_d = np.full((NCORES, NB, P), -1.0, dtype=np.float32)
    diag_e = np.zeros((NCORES, NB, P), dtype=np.float32)
    for k in range(NCORES):
        for b in range(NB):
            node_lo = k * SH + b * 128
            nreal = min(128, SH - b * 128)
            diag_d[k, b, :nreal] = np.arange(nreal, dtype=np.float32)
            diag_e[k, b, :nreal] = self_en[node_lo : node_lo + nreal]
    dstoff = np.concatenate([dstoff, diag_d], axis=1)
    enormt = np.concatenate([enormt, diag_e], axis=1)

    dstoff = np.ascontiguousarray(dstoff.transpose(0, 2, 1))
    enormt = np.ascontiguousarray(enormt.transpose(0, 2, 1))
    return (
        [int(g) for g in G],
        [int(g) for g in Glo],
        [int(g) for g in Ghi],
        NG,
        idx16,
        dstoff,
        enormt,
    )


# ---------------------------------------------------------------- device program

def _build_program(G, Glo, Ghi, NG, num_devices=NCORES):
    """Build the per-core Bass program (SPMD shared across cores)."""
    nc = bacc.Bacc(
        "TRN2", target_bir_lowering=False, debug=False, num_devices=num_devices
    )

    # ---------- I/O
    xt_d = nc.dram_tensor("xt", [NFEAT_P, SHP], DENSE_DT, kind="ExternalInput")
    w1_d = nc.dram_tensor("w1", [NFEAT_P, H1], DENSE_DT, kind="ExternalInput")
    w2_d = nc.dram_tensor("w2", [H1, H2], F16, kind="ExternalInput")
    wi1_d = nc.dram_tensor("wi1", [H2, PRO], F16, kind="ExternalInput")
    wi2_d = nc.dram_tensor("wi2", [PRO, H2], F16, kind="ExternalInput")
    wc1_d = nc.dram_tensor("wc1", [H2, PRO], F16, kind="ExternalInput")
    wc2_d = nc.dram_tensor("wc2", [PRO, NCLS_P], F16, kind="ExternalInput")
    b1b_d = nc.dram_tensor("b1b", [P, H1], F16, kind="ExternalInput")
    b2b_d = nc.dram_tensor("b2b", [P, H2], F16, kind="ExternalInput")
    bi1c_d = nc.dram_tensor("bi1c", [P, 4], F32, kind="ExternalInput")
    bc1c_d = nc.dram_tensor("bc1c", [P, 4], F32, kind="ExternalInput")
    bi2b_d = nc.dram_tensor("bi2b", [P, H2], F32, kind="ExternalInput")
    bc2b_d = nc.dram_tensor("bc2b", [P, NCLS_P], F32, kind="ExternalInput")
    iden_d = nc.dram_tensor("iden", [P, P], F16, kind="ExternalInput")
    iota_d = nc.dram_tensor("iota", [P, P], F16, kind="ExternalInput")
    idx16_d = nc.dram_tensor("idx16", [P, 8 * NG], mybir.dt.int16, kind="ExternalInput")
    dsto_d = nc.dram_tensor("dstoff", [P, NG + NB], F32, kind="ExternalInput")
    enrm_d = nc.dram_tensor("enorm", [P, NG + NB], F32, kind="ExternalInput")

    z_out = nc.dram_tensor("z_out", [SHP, H2], F32, kind="ExternalOutput")
    c_out = nc.dram_tensor("c_out", [SHP, NCLS_P], F32, kind="ExternalOutput")

    g0 = [0]
    for g in G:
        g0.append(g0[-1] + g)

    rg = [list(range(num_devices))]

    with tile.TileContext(nc) as tc:
        with (
            tc.tile_pool(name="const", bufs=1) as cpool,
            tc.tile_pool(name="dram", bufs=1, space="DRAM") as dpool,
        ):
            # ---------- DRAM intermediates
            xw_sh = dpool.tile([SHP, H1], TABLE_DT)
            xw_full = dpool.tile([NCORES * SHP, H1], TABLE_DT, addr_space="Shared")
            hw2_sh = dpool.tile([SHP, H2], TABLE_DT)
            hw2_full = dpool.tile([NCORES * SHP, H2], TABLE_DT, addr_space="Shared")

            # ---------- constants into SBUF
            def cload(dram, shape, dt, name):
                t = cpool.tile(shape, dt, name=name)
                nc.sync.dma_start(out=t[:], in_=dram[:])
                return t

            def cload_kt(dram, n_kt, ncols, dt, name):
                # [n_kt*128, ncols] DRAM -> [128, n_kt*ncols] SBUF (k-tiles packed)
                t = cpool.tile([P, n_kt * ncols], dt, name=name)
                for kt in range(n_kt):
                    nc.sync.dma_start(
                        out=t[:, kt * ncols : (kt + 1) * ncols],
                        in_=dram[kt * P : (kt + 1) * P, :],
                    )
                return t

            w1_s = cload_kt(w1_d, 4, H1, DENSE_DT, "w1_s")
            w2_s = cload_kt(w2_d, 2, H2, F16, "w2_s")
            wi1_s = cload(wi1_d, [H2, PRO], F16, "wi1_s")
            wi2_s = cload_kt(wi2_d, 4, H2, F16, "wi2_s")
            wc1_s = cload(wc1_d, [H2, PRO], F16, "wc1_s")
            wc2_s = cload_kt(wc2_d, 4, NCLS_P, F16, "wc2_s")
            b1b_s = cload(b1b_d, [P, H1], F16, "b1b_s")
            b2b_s = cload(b2b_d, [P, H2], F16, "b2b_s")
            bi1c_s = cload(bi1c_d, [P, 4], F32, "bi1c_s")
            bc1c_s = cload(bc1c_d, [P, 4], F32, "bc1c_s")
            bi2b_s = cload(bi2b_d, [P, H2], F32, "bi2b_s")
            bc2b_s = cload(bc2b_d, [P, NCLS_P], F32, "bc2b_s")
            iden_s = cload(iden_d, [P, P], F16, "iden_s")
            iota_s = cload(iota_d, [P, P], F16, "iota_s")
            idx16_s = cload(idx16_d, [P, 8 * NG], mybir.dt.int16, "idx16_s")
            dsto_s = cload(dsto_d, [P, NG + NB], F32, "dsto_s")
            enrm_s = cload(enrm_d, [P, NG + NB], F32, "enrm_s")

            # w1 views: k-tile kt -> [128, H1] at rows kt*128
            # ---------- Phase 1: xw = x @ W1  (fp32r), store fp16
            with (
                tc.tile_pool(name="xpool", bufs=1) as xpool,
                tc.tile_pool(name="p1sb", bufs=3) as p1sb,
                tc.tile_pool(name="p1ps", bufs=2, space="PSUM") as p1ps,
            ):
                xsb = xpool.tile([P, 4 * SHP], DENSE_DT, name="xsb")
                for kt in range(4):
                    nc.sync.dma_start(
                        out=xsb[:, kt * SHP : (kt + 1) * SHP],
                        in_=xt_d[kt * P : (kt + 1) * P, :],
                    )
                for nb in range(NB):
                    ps = p1ps.tile([P, H1], F32, name="xw_ps")
                    for kt in range(4):
                        lhsT = xsb[:, kt * SHP + nb * P : kt * SHP + (nb + 1) * P]
                        nc.tensor.matmul(
                            ps[:],
                            lhsT,
                            w1_s[:, kt * H1 : (kt + 1) * H1],
                            start=(kt == 0),
                            stop=(kt == 3),
                        )
                    xw_t = p1sb.tile([P, H1], TABLE_DT, name="xw_t")
                    nc.scalar.activation(xw_t[:], ps[:], ACTF.Copy)
                    nc.sync.dma_start(
                        out=xw_sh[nb * P : (nb + 1) * P, :], in_=xw_t[:]
                    )

            # ---------- Phase 2: AllGather xw
            if STAGE >= 2:
                nc.gpsimd.collective_compute(
                "AllGather",
                ALU.bypass,
                    replica_groups=rg,
                    ins=[xw_sh[:]],
                    outs=[xw_full[:]],
                )

            # ---------- Phase 3: layer-1 aggregation + ReLU + @W2, store fp16
            if STAGE >= 3:
              with (
                tc.tile_pool(name="p3gt", bufs=8) as p3gt,
                tc.tile_pool(name="p3sb", bufs=3) as p3sb,
                tc.tile_pool(name="p3oh", bufs=12) as p3oh,
                tc.tile_pool(name="p3ps", bufs=2, space="PSUM") as p3ps,
                tc.tile_pool(name="p3pt", bufs=2, space="PSUM") as p3pt,
            ):
                HALF = NCORES * SHP // 2
                for nb in range(NB):
                    g_lo, g_n = g0[nb], G[nb]
                    ps = p3ps.tile([P, H1], F32, name="agg_ps")
                    gt = p3gt.tile([P, g_n * H1], TABLE_DT, name="gt", tag="gt")
                    for gh, goff, tab in (
                        (Glo[nb], 0, xw_full[0:HALF, :]),
                        (Ghi[nb], Glo[nb], xw_full[HALF:, :]),
                    ):
                        if gh == 0:
                            continue
                        nidx = gh * P
                        nc.gpsimd.dma_gather(
                            out_ap=gt[:, goff * H1 : (goff + gh) * H1].rearrange(
                                "p (j d) -> p j d", d=H1
                            ),
                            in_ap=tab,
                            idxs_ap=idx16_s[
                                :, 8 * (g_lo + goff) : 8 * (g_lo + goff + gh)
                            ],
                            num_idxs=nidx,
                            num_idxs_reg=nidx,
                            elem_size=H1,
                            single_packet=False,
                        )
                    for g in range(g_n):
                        oh = p3oh.tile([P, P], TABLE_DT, name="oh", tag="oh")
                        nc.vector.tensor_scalar(
                            out=oh[:],
                            in0=iota_s[:],
                            scalar1=dsto_s[:, g_lo + g : g_lo + g + 1],
                            scalar2=enrm_s[:, g_lo + g : g_lo + g + 1],
                            op0=ALU.is_equal,
                            op1=ALU.mult,
                        )
                        nc.tensor.matmul(
                            ps[:],
                            oh[:],
                            gt[:, g * H1 : (g + 1) * H1],
                            start=(g == 0),
                            stop=False,
                        )
                    gtd = p3gt.tile([P, H1], TABLE_DT, name="gtd", tag="gtd")
                    nc.sync.dma_start(
                        out=gtd[:], in_=xw_sh[nb * P : (nb + 1) * P, :]
                    )
                    ohd = p3oh.tile([P, P], TABLE_DT, name="ohd", tag="oh")
                    nc.vector.tensor_scalar(
                        out=ohd[:],
                        in0=iota_s[:],
                        scalar1=dsto_s[:, NG + nb : NG + nb + 1],
                        scalar2=enrm_s[:, NG + nb : NG + nb + 1],
                        op0=ALU.is_equal,
                        op1=ALU.mult,
                    )
                    nc.tensor.matmul(ps[:], ohd[:], gtd[:], start=False, stop=False)
                    nc.tensor.matmul(
                        ps[:], iden_s[:], b1b_s[:], start=False, stop=True
                    )
                    h1 = p3sb.tile([P, H1], TABLE_DT, name="h1", tag="h1")
                    nc.scalar.activation(h1[:], ps[:], ACTF.Relu)
                    # transpose h1 -> h1T (2x 128x128), then @ W2
                    ps2 = p3ps.tile([P, H2], F32, name="hw2_ps", tag="hw2_ps")
                    for half in range(2):
                        pt = p3pt.tile([P, P], TABLE_DT, name="tps", tag="tps")
                        nc.tensor.transpose(
                            pt[:], h1[:, half * P : (half + 1) * P], iden_s[:]
                        )
                        h1t = p3sb.tile([P, P], TABLE_DT, name="h1t", tag="h1t")
                        nc.vector.tensor_copy(h1t[:], pt[:])
                        nc.tensor.matmul(
                            ps2[:],
                            h1t[:],
                            w2_s[:, half * H2 : (half + 1) * H2],
                            start=(half == 0),
                            stop=(half == 1),
                        )
                    hw2t = p3sb.tile([P, H2], TABLE_DT, name="hw2t", tag="hw2t")
                    nc.vector.tensor_copy(hw2t[:], ps2[:])
                    nc.sync.dma_start(
                        out=hw2_sh[nb * P : (nb + 1) * P, :], in_=hw2t[:]
                    )

            # ---------- Phase 4: AllGather hw2
            if STAGE >= 4:
                nc.gpsimd.collective_compute(
                "AllGather",
                ALU.bypass,
                    replica_groups=rg,
                    ins=[hw2_sh[:]],
                    outs=[hw2_full[:]],
                )

            # ---------- Phase 5: layer-2 aggregation -> h2T resident [128, SHP]
            if STAGE >= 5:
              with tc.tile_pool(name="h2pool", bufs=1) as h2pool:
                h2T = h2pool.tile([P, SHP], TABLE_DT, name="h2T")
                with (
                    tc.tile_pool(name="p5gt", bufs=8) as p5gt,
                    tc.tile_pool(name="p5sb", bufs=3) as p5sb,
                    tc.tile_pool(name="p5oh", bufs=12) as p5oh,
                    tc.tile_pool(name="p5ps", bufs=2, space="PSUM") as p5ps,
                    tc.tile_pool(name="p5pt", bufs=2, space="PSUM") as p5pt,
                ):
                    HALF2 = NCORES * SHP // 2
                    for nb in range(NB):
                        g_lo, g_n = g0[nb], G[nb]
                        ps = p5ps.tile([P, H2], F32, name="agg2_ps")
                        gt = p5gt.tile([P, g_n * H2], TABLE_DT, name="gt2", tag="gt2")
                        for gh, goff, tab in (
                            (Glo[nb], 0, hw2_full[0:HALF2, :]),
                            (Ghi[nb], Glo[nb], hw2_full[HALF2:, :]),
                        ):
                            if gh == 0:
                                continue
                            nidx = gh * P
                            nc.gpsimd.dma_gather(
                                out_ap=gt[:, goff * H2 : (goff + gh) * H2].rearrange(
                                    "p (j d) -> p j d", d=H2
                                ),
                                in_ap=tab,
                                idxs_ap=idx16_s[
                                    :, 8 * (g_lo + goff) : 8 * (g_lo + goff + gh)
                                ],
                                num_idxs=nidx,
                                num_idxs_reg=nidx,
                                elem_size=H2,
                                single_packet=False,
                            )
                        for g in range(g_n):
                            oh = p5oh.tile([P, P], TABLE_DT, name="oh2", tag="oh2")
                            nc.vector.tensor_scalar(
                                out=oh[:],
                                in0=iota_s[:],
                                scalar1=dsto_s[:, g_lo + g : g_lo + g + 1],
                                scalar2=enrm_s[:, g_lo + g : g_lo + g + 1],
                                op0=ALU.is_equal,
                                op1=ALU.mult,
                            )
                            nc.tensor.matmul(
                                ps[:],
                                oh[:],
                                gt[:, g * H2 : (g + 1) * H2],
                                start=(g == 0),
                                stop=False,
                            )
                        gtd = p5gt.tile([P, H2], TABLE_DT, name="gtd2", tag="gtd2")
                        nc.sync.dma_start(
                            out=gtd[:], in_=hw2_sh[nb * P : (nb + 1) * P, :]
                        )
                        ohd = p5oh.tile([P, P], TABLE_DT, name="ohd2", tag="oh2")
                        nc.vector.tensor_scalar(
                            out=ohd[:],
                            in0=iota_s[:],
                            scalar1=dsto_s[:, NG + nb : NG + nb + 1],
                            scalar2=enrm_s[:, NG + nb : NG + nb + 1],
                            op0=ALU.is_equal,
                            op1=ALU.mult,
                        )
                        nc.tensor.matmul(ps[:], ohd[:], gtd[:], start=False, stop=False)
                        nc.tensor.matmul(
                            ps[:], iden_s[:], b2b_s[:], start=False, stop=True
                        )
                        h2 = p5sb.tile([P, H2], TABLE_DT, name="h2", tag="h2")
                        nc.scalar.activation(h2[:], ps[:], ACTF.Relu)
                        pt = p5pt.tile([P, P], TABLE_DT, name="tps2", tag="tps2")
                        nc.tensor.transpose(pt[:], h2[:], iden_s[:])
                        nc.vector.tensor_copy(
                            h2T[:, nb * P : (nb + 1) * P], pt[:]
                        )

                # ---------- Phase 6: heads
                chunks = []
                c0 = 0
                while STAGE >= 6 and c0 < SHP:
                    w = min(512, SHP - c0)
                    chunks.append((c0, w))
                    c0 += w
                with (
                    tc.tile_pool(name="p6sb", bufs=3) as p6sb,
                    tc.tile_pool(name="p6ps", bufs=3, space="PSUM") as p6ps,
                    tc.tile_pool(name="p6pz", bufs=2, space="PSUM") as p6pz,
                ):
                    for c0, w in chunks:
                        rhs = h2T[:, c0 : c0 + w]
                        t1T = []
                        u1T = []
                        for i in range(4):
                            pst = p6ps.tile([P, w], F32, name="t1_ps", tag="hps")
                            nc.tensor.matmul(
                                pst[:],
                                wi1_s[:, i * P : (i + 1) * P],
                                rhs,
                                start=True,
                                stop=True,
                            )
                            t1 = p6sb.tile(
                                [P, w], TABLE_DT, name="t1t", tag=f"t1t{i}", bufs=2
                            )
                            nc.scalar.activation(
                                t1[:], pst[:], ACTF.Relu,
                                bias=bi1c_s[:, i : i + 1],
                            )
                            t1T.append(t1)
                        for i in range(4):
                            psu = p6ps.tile([P, w], F32, name="u1_ps", tag="hps")
                            nc.tensor.matmul(
                                psu[:],
                                wc1_s[:, i * P : (i + 1) * P],
                                rhs,
                                start=True,
                                stop=True,
                            )
                            u1 = p6sb.tile(
                                [P, w], TABLE_DT, name="u1t", tag=f"u1t{i}", bufs=2
                            )
                            nc.scalar.activation(
                                u1[:], psu[:], ACTF.Relu,
                                bias=bc1c_s[:, i : i + 1],
                            )
                            u1T.append(u1)
                        for s in range(w // P):
                            row0 = c0 + s * P
                            # ---- z head
                            pz = p6pz.tile([P, H2], F32, name="zi_ps", tag="zps")
                            for i in range(4):
                                nc.tensor.matmul(
                                    pz[:],
                                    t1T[i][:, s * P : (s + 1) * P],
                                    wi2_s[:, i * H2 : (i + 1) * H2],
                                    start=(i == 0),
                                    stop=(i == 3),
                                )
                            zi = p6sb.tile([P, H2], F32, name="zi", tag="zi")
                            nc.vector.tensor_add(zi[:], pz[:], bi2b_s[:])
                            if STAGE >= 8:
                                sq = p6sb.tile([P, H2], F32, name="sq", tag="sq")
                                ss = p6sb.tile([P, 1], F32, name="ss", tag="ss")
                                nc.vector.tensor_mul(sq[:], zi[:], zi[:])
                                nc.vector.tensor_reduce(
                                    out=ss[:], in_=sq[:],
                                    axis=mybir.AxisListType.X, op=ALU.add,
                                )
                                nrm = p6sb.tile([P, 1], F32, name="nrm", tag="nrm")
                                nc.scalar.activation(nrm[:], ss[:], ACTF.Sqrt)
                                nc.vector.tensor_scalar(
                                    out=nrm[:], in0=nrm[:],
                                    scalar1=1e-12, scalar2=None, op0=ALU.max,
                                )
                                rn = p6sb.tile([P, 1], F32, name="rn", tag="rn")
                                nc.vector.reciprocal(rn[:], nrm[:])
                                zt = p6sb.tile([P, H2], F32, name="zt", tag="zt")
                                nc.vector.tensor_scalar(
                                    out=zt[:], in0=zi[:],
                                    scalar1=rn[:], scalar2=None, op0=ALU.mult,
                                )
                            else:
                                zt = zi
                            nc.sync.dma_start(
                                out=z_out[row0 : row0 + P, :], in_=zt[:]
                            )
                            # ---- c head
                            pl = p6pz.tile([P, NCLS_P], F32, name="lg_ps", tag="lps")
                            for i in range(4):
                                nc.tensor.matmul(
                                    pl[:],
                                    u1T[i][:, s * P : (s + 1) * P],
                                    wc2_s[:, i * NCLS_P : (i + 1) * NCLS_P],
                                    start=(i == 0),
                                    stop=(i == 3),
                                )
                            lg = p6sb.tile([P, NCLS_P], F32, name="lg", tag="lg")
                            nc.vector.tensor_add(lg[:], pl[:], bc2b_s[:])
                            if STAGE >= 9:
                                rm = p6sb.tile([P, 1], F32, name="rm", tag="rm")
                                nc.vector.tensor_reduce(
                                    out=rm[:], in_=lg[:],
                                    axis=mybir.AxisListType.X, op=ALU.max,
                                )
                                nrm2 = p6sb.tile([P, 1], F32, name="nrm2", tag="nrm2")
                                nc.vector.tensor_scalar(
                                    out=nrm2[:], in0=rm[:],
                                    scalar1=-1.0, scalar2=None, op0=ALU.mult,
                                )
                                ex = p6sb.tile([P, NCLS_P], F32, name="ex", tag="ex")
                                es = p6sb.tile([P, 1], F32, name="es", tag="es")
                                nc.scalar.activation(
                                    ex[:], lg[:], ACTF.Exp, bias=nrm2[:],
                                )
                                nc.vector.tensor_reduce(
                                    out=es[:], in_=ex[:],
                                    axis=mybir.AxisListType.X, op=ALU.add,
                                )
                                res = p6sb.tile([P, 1], F32, name="res", tag="res")
                                nc.vector.reciprocal(res[:], es[:])
                                ct = p6sb.tile([P, NCLS_P], F32, name="ct", tag="ct")
                                nc.vector.tensor_scalar(
                                    out=ct[:], in0=ex[:],
                                    scalar1=res[:], scalar2=None, op0=ALU.mult,
                                )
                            else:
                                ct = lg
                            nc.sync.dma_start(
                                out=c_out[row0 : row0 + P, :], in_=ct[:]
                            )

            if STAGE < 6:
                _dbg = cpool.tile([P, H2], F32, name="_dbg")
                nc.vector.tensor_copy(_dbg[:], iota_s[:, :H2])
                nc.sync.dma_start(out=z_out[0:P, :], in_=_dbg[:])
                nc.sync.dma_start(out=c_out[0:P, :], in_=_dbg[:, :NCLS_P])

    nc.compile()
    return nc


# ---------------------------------------------------------------- host driver

_CACHE = {}


def _get_program(G, Glo, Ghi, NG):
    key = (tuple(Glo), tuple(Ghi), NG)
    if key not in _CACHE:
        _CACHE[key] = _build_program(G, Glo, Ghi, NG)
    return _CACHE[key]


def _prepare(x, edge_index, W1, b1, W2, b2, Wi1, bi1, Wi2, bi2, Wc1, bc1, Wc2, bc2):
    x = np.asarray(x, dtype=np.float32)
    edge_index = np.asarray(edge_index)

    G, Glo, Ghi, NG, idx16, dstoff, enormt = _preprocess(edge_index)
    nc = _get_program(G, Glo, Ghi, NG)

    # shared (replicated) tensors
    w1 = np.zeros((NFEAT_P, H1), np.float32)
    w1[:NFEAT] = np.asarray(W1, np.float32)
    w2 = np.asarray(W2, np.float16)
    wi1 = np.asarray(Wi1, np.float16)
    wi2 = np.asarray(Wi2, np.float16)
    wc1 = np.asarray(Wc1, np.float16)
    wc2 = np.zeros((PRO, NCLS_P), np.float16)
    wc2[:, :NCLS] = np.asarray(Wc2, np.float16)
    b1b = np.broadcast_to(np.asarray(b1, np.float16), (P, H1)).copy()
    b2b = np.broadcast_to(np.asarray(b2, np.float16), (P, H2)).copy()
    bi1c = np.ascontiguousarray(
        np.broadcast_to(np.asarray(bi1, np.float32).reshape(4, P).T, (P, 4))
    )
    bc1c = np.ascontiguousarray(
        np.broadcast_to(np.asarray(bc1, np.float32).reshape(4, P).T, (P, 4))
    )
    bi2b = np.broadcast_to(np.asarray(bi2, np.float32), (P, H2)).copy()
    bc2b = np.full((P, NCLS_P), -30000.0, np.float32)
    bc2b[:, :NCLS] = np.asarray(bc2, np.float32)
    iden = np.eye(P, dtype=np.float16)
    iota = np.broadcast_to(np.arange(P, dtype=np.float16), (P, P)).copy()

    in_maps = []
    for k in range(NCORES):
        xt = np.zeros((NFEAT_P, SHP), np.float32)
        xt[:NFEAT, :SH] = x[k * SH : (k + 1) * SH].T
        in_maps.append(
            {
                "xt": xt,
                "w1": w1, "w2": w2, "wi1": wi1, "wi2": wi2,
                "wc1": wc1, "wc2": wc2,
                "b1b": b1b, "b2b": b2b, "bi1c": bi1c, "bc1c": bc1c,
                "bi2b": bi2b, "bc2b": bc2b,
                "iden": iden, "iota": iota,
                "idx16": idx16[k], "dstoff": dstoff[k], "enorm": enormt[k],
            }
        )

    return nc, in_maps


def _postprocess(results):
    z = np.concatenate([results[k]["z_out"][:SH] for k in range(NCORES)])
    c = np.concatenate([results[k]["c_out"][:SH, :NCLS] for k in range(NCORES)])
    return z.astype(np.float32), c.astype(np.float32)


def kernel(**inputs):
    nc, in_maps = _prepare(**inputs)
    res = run_bass_kernel_spmd(nc, in_maps, core_ids=list(range(NCORES)))
    kernel.last_results = res
    return _postprocess(res.results)
